# revision 1
# baseline (speedup 1.0000x reference)
"""Trainium2 Bass kernel: batched ChebConv GNN with L1-distance adjacency.

Pipeline per sample (N=512 nodes, F=625 features):
  1. Sort nodes by attention (host). All pairs with |att_i-att_j| <= 0.05
     then lie within a rank band |i-j| <= w (w computed exactly on host).
  2. Banded pairwise L1 distances on device using the exact identity
     sum_f |a-b| = 2*sum_f max(a,b) - S_i - S_j  (S = row sums), computed
     with one fused DVE op per (offset, feature-chunk) + a PE ones-matmul
     for the cross-partition feature reduction.
  3. Threshold masks -> banded adjacency -> scattered to a DRAM matrix via
     diagonal-stride DMAs (identity on the diagonal).
  4. Degree-normalized ChebConv x2 as float32r PE matmuls in transposed
     layouts (the dinv column scaling is commuted through the weight
     matmuls so it is always a cheap per-partition row scaling).
Data parallel over batch: 16 samples, 8 cores, 2 samples/core.
"""

import numpy as np
from contextlib import ExitStack

B, N = 16, 512
F, FH = 625, 937
FCH, NFCH = 125, 5  # feature chunks: 5 x 125 = 625
NCORES = 8
SPB = B // NCORES  # samples per core
DIST_THRESH, ATT_THRESH = 180.0, 0.05
DCH = 48  # band offsets per PSUM group (psM tile base partition stays 0)

# FH row blocks (7x128 + 41)
FH_BLOCKS = [(o, min(128, FH - o)) for o in range(0, FH, 128)]

_prog_cache = {}


def _build_program(w, mm="f32r", reps=1):
    """Build the SPMD Bass program for band half-width w. Returns (nc, meta).

    mm: dtype mode for matmul operands: "fp32" (4 cyc/row, exact) or
    "f32r" (1 cyc/row at N>=512, reduced-precision PE input rounding).
    """
    import concourse.bass as bass
    import concourse.bacc as bacc
    import concourse.mybir as mybir
    import concourse.tile as tile
    from concourse.masks import make_identity

    dt = mybir.dt
    fp = dt.float32
    fr = dt.float32r
    mdt = fr if mm == "f32r" else fp
    AF = mybir.ActivationFunctionType
    OP = mybir.AluOpType
    AX = mybir.AxisListType
    AP = bass.AP

    padw = ((w + 7) // 8) * 8
    WROW = N + padw  # padded row width for xpt/attp/scratch

    nc = bacc.Bacc()
    xp_p = nc.declare_dram_parameter("xp", [SPB, N, F], mdt, isOutput=False)
    xpt_p = nc.declare_dram_parameter("xpt", [SPB, F, WROW], mdt, isOutput=False)
    attp_p = nc.declare_dram_parameter("attp", [SPB, WROW], fp, isOutput=False)
    w1_p = nc.declare_dram_parameter("w1", [2, F, FH], mdt, isOutput=False)
    b1_p = nc.declare_dram_parameter("b1", [FH], fp, isOutput=False)
    w2_p = nc.declare_dram_parameter("w2", [2, FH, F], mdt, isOutput=False)
    b2_p = nc.declare_dram_parameter("b2", [F], fp, isOutput=False)
    out_p = nc.declare_dram_parameter("outT", [SPB, F, N], fp, isOutput=True)
    ones_p = nc.declare_dram_parameter("c_ones", [128, 1], mdt, isOutput=False)
    onesrow_p = nc.declare_dram_parameter("c_onesrow", [1, N], mdt, isOutput=False)
    estep_p = nc.declare_dram_parameter("c_estep", [FCH, 95], mdt, isOutput=False)
    zeros_p = nc.declare_dram_parameter("c_zeros", [128, WROW], mdt, isOutput=False)

    # internal DRAM scratch, one set per sample slot
    a_scr = [nc.dram_tensor(f"a_scr{b}", [WROW * WROW], mdt) for b in range(SPB)]
    s_scr = [nc.dram_tensor(f"s_scr{b}", [WROW], fp) for b in range(SPB)]
    d_scr = [nc.dram_tensor(f"d_scr{b}", [N], fp) for b in range(SPB)]

    with tile.TileContext(nc) as tc, ExitStack() as ctx:
        cst = ctx.enter_context(tc.tile_pool(name="cst", bufs=1))
        xtp = ctx.enter_context(tc.tile_pool(name="xtp", bufs=1))
        xpp = ctx.enter_context(tc.tile_pool(name="xpp", bufs=1))  # xn: cheb phase only
        mxp = ctx.enter_context(tc.tile_pool(name="mxp", bufs=2))
        bnd = ctx.enter_context(tc.tile_pool(name="bnd", bufs=1))
        amp = ctx.enter_context(tc.tile_pool(name="amp", bufs=1))
        acp = ctx.enter_context(tc.tile_pool(name="acp", bufs=1))
        wsp = ctx.enter_context(tc.tile_pool(name="wsp", bufs=1))
        otp = ctx.enter_context(tc.tile_pool(name="otp", bufs=1))
        psp = ctx.enter_context(tc.tile_pool(name="psp", bufs=2, space="PSUM"))
        psb = ctx.enter_context(tc.tile_pool(name="psb", bufs=2, space="PSUM"))

        ones = cst.tile([128, 1], mdt, tag="ones")
        nc.sync.dma_start(ones[:], ones_p[:, :])
        ident = cst.tile([128, 128], fp, tag="ident")
        make_identity(nc, ident[:])
        onesrow = cst.tile([1, N], mdt, tag="onesrow")
        nc.sync.dma_start(onesrow[:], onesrow_p[:, :])
        # staircase selector: estep[:, 47-di : 47-di+dn] is a [FCH, dn]
        # matrix whose only nonzero column is column di (all ones) -> matmul
        # with it as lhsT reduces partitions into PSUM row di (base 0)
        estep = cst.tile([FCH, 95], mdt, tag="estep")
        nc.sync.dma_start(estep[:], estep_p[:, :])

        import os as _os
        BD = 4  # band offsets per DVE instruction
        use_gpsimd = _os.environ.get("KERNEL_GPSIMD", "0") == "1"
        bcount = [0]

        # ---- setup phase (once per call): resident weights, biases,
        #      adjacency scratch init, S padding
        w1t = [[wsp.tile([FCH, FH], mdt, tag=f"w1t{k_}{c_}", name=f"w1t{k_}{c_}")
                for c_ in range(NFCH)] for k_ in range(2)]
        for k_ in range(2):
            for c_ in range(NFCH):
                nc.sync.dma_start(w1t[k_][c_][:],
                                  w1_p[k_, c_ * FCH:(c_ + 1) * FCH, :])
        w2t = [[wsp.tile([128, F], mdt, tag=f"w2t{k_}{j_}", name=f"w2t{k_}{j_}")
                for j_ in range(len(FH_BLOCKS))] for k_ in range(2)]
        for k_ in range(2):
            for j_, (ko, kp) in enumerate(FH_BLOCKS):
                nc.sync.dma_start(w2t[k_][j_][:kp, :],
                                  w2_p[k_, ko:ko + kp, :])
        b1t = [wsp.tile([128, 1], fp, tag=f"b1t{j_}", name=f"b1t{j_}")
               for j_ in range(len(FH_BLOCKS))]
        for j_, (mo, mp_) in enumerate(FH_BLOCKS):
            nc.sync.dma_start(b1t[j_][:mp_, :], b1_p[mo:mo + mp_])
        b2t = [wsp.tile([FCH, 1], fp, tag=f"b2t{m_}", name=f"b2t{m_}")
               for m_ in range(NFCH)]
        for m_ in range(NFCH):
            nc.sync.dma_start(b2t[m_][:], b2_p[m_ * FCH:(m_ + 1) * FCH])
        for b in range(SPB):
            ad, sd = a_scr[b], s_scr[b]
            for t in range(4):
                nc.sync.dma_start(AP(ad, t * 128 * WROW, [[1, 128 * WROW]]),
                                  AP(zeros_p, 0, [[1, 128 * WROW]]))
            nc.sync.dma_start(AP(ad, 0, [[WROW + 1, N]]), onesrow[:])
            nc.sync.dma_start(AP(sd, N, [[1, WROW - N]]),
                              AP(zeros_p, 0, [[1, WROW - N]]).bitcast(fp))

        rep_cm = tc.For_i(0, reps, 1) if reps > 1 else None
        if rep_cm is not None:
            rep_cm.__enter__()

        # ================= phase 1: input loads + row sums =================
        xt_all, srow_all = [], []
        for b in range(SPB):
            sd = s_scr[b]
            xt = [xtp.tile([FCH, WROW], mdt, tag=f"xt{b}{c}",
                           name=f"xt{b}{c}") for c in range(NFCH)]
            for c in range(NFCH):
                nc.sync.dma_start(xt[c][:], xpt_p[b, c * FCH:(c + 1) * FCH, :])
            xt_all.append(xt)
            psS = psb.tile([1, N], fp, tag="psS")
            for c in range(NFCH):
                nc.tensor.matmul(psS[:], ones[:FCH, :], xt[c][:, :N],
                                 start=(c == 0), stop=(c == NFCH - 1))
            srow = bnd.tile([1, N], fp, tag=f"srow{b}", name=f"srow{b}")
            nc.scalar.copy(srow[:], psS[:])
            nc.sync.dma_start(AP(sd, 0, [[1, N]]), srow[:])

        # ============ phase 2/3: bands and chebs, software-pipelined =======
        at_all = [None] * SPB

        def gen_band(b):
            ad, sd, dd = a_scr[b], s_scr[b], d_scr[b]
            xt = xt_all[b]
            d0 = 1
            while d0 <= w:
                dn = min(DCH, w - d0 + 1)
                psM = psb.tile([dn, N], fp, tag="psM", name="psM")
                nbatches = (dn + BD - 1) // BD
                for bi in range(nbatches):
                    db0 = bi * BD
                    nb = min(BD, dn - db0)
                    for c in range(NFCH):
                        mxb = mxp.tile([FCH, BD * N], mdt, tag="mx",
                                       name="mxb")
                        base = xt[c][:, 0:N]
                        in0 = bass.AP(base.tensor, base.offset,
                                      [list(base.ap[0]), [0, nb], [1, N]])
                        in1 = bass.AP(base.tensor, base.offset + d0 + db0,
                                      [list(base.ap[0]), [1, nb], [1, N]])
                        nc.vector.tensor_tensor(
                            out=mxb[:, :nb * N], in0=in0, in1=in1, op=OP.max)
                        for j in range(nb):
                            di = db0 + j
                            nc.tensor.matmul(
                                psM[:, :],
                                estep[:, 47 - di:47 - di + dn],
                                mxb[:, j * N:(j + 1) * N],
                                start=(di == 0 and c == 0),
                                stop=(di == dn - 1 and c == NFCH - 1))
                        yield
                # epilogue: D = 2M - S_i - S_{i+d}; masks; scatter
                sshift = bnd.tile([dn, N], fp, tag="sshift", name="sshift")
                nc.sync.dma_start(sshift[:], AP(sd, d0, [[1, dn], [1, N]]))
                sb_t = bnd.tile([dn, N], fp, tag="sb", name="sb_t")
                nc.sync.dma_start(sb_t[:], AP(sd, 0, [[0, dn], [1, N]]))
                ashift = bnd.tile([dn, N], fp, tag="ashift", name="ashift")
                nc.sync.dma_start(ashift[:],
                                  AP(attp_p, b * WROW + d0, [[1, dn], [1, N]]))
                ab_t = bnd.tile([dn, N], fp, tag="ab", name="ab_t")
                nc.sync.dma_start(ab_t[:],
                                  AP(attp_p, b * WROW, [[0, dn], [1, N]]))
                nc.vector.scalar_tensor_tensor(
                    out=sb_t[:], in0=sb_t[:], scalar=DIST_THRESH, in1=sshift[:],
                    op0=OP.add, op1=OP.add)
                nc.vector.scalar_tensor_tensor(
                    out=sshift[:], in0=psM[:], scalar=2.0, in1=sb_t[:],
                    op0=OP.mult, op1=OP.is_le)
                nc.vector.tensor_sub(ashift[:], ashift[:], ab_t[:])
                nc.vector.tensor_scalar(ab_t[:], ashift[:], ATT_THRESH, None,
                                        op0=OP.is_le)
                abnd = bnd.tile([dn, N], mdt, tag="abnd", name="abnd")
                nc.vector.tensor_mul(abnd[:], sshift[:], ab_t[:])
                nc.sync.dma_start(AP(ad, d0, [[1, dn], [WROW + 1, N]]),
                                  abnd[:])
                nc.sync.dma_start(AP(ad, d0 * WROW, [[WROW, dn], [WROW + 1, N]]),
                                  abnd[:])
                d0 += dn
                yield

            at = [amp.tile([128, N], mdt, tag=f"at{b}{t}", name=f"at{b}{t}")
                  for t in range(4)]
            for t in range(4):
                nc.sync.dma_start(at[t][:],
                                  AP(ad, t * 128 * WROW, [[WROW, 128], [1, N]]))
            for t in range(4):
                deg = bnd.tile([128, 1], fp, tag="deg", name="deg")
                nc.vector.tensor_reduce(deg[:], at[t][:], axis=AX.X, op=OP.add)
                dv = bnd.tile([128, 1], fp, tag="dv", name="dv")
                nc.vector.reciprocal(dv[:], deg[:])
                nc.sync.dma_start(AP(dd, t * 128, [[1, 128]]), dv[:])
            dinvB = amp.tile([128, N], fp, tag=f"dinvB{b}", name=f"dinvB{b}")
            nc.sync.dma_start(dinvB[:], AP(dd, 0, [[0, 128], [1, N]]))
            # at' = (A+I) diag(1/deg): both Cheb terms use it directly
            for t in range(4):
                nc.vector.tensor_mul(at[t][:], at[t][:], dinvB[:])
            at_all[b] = at
            yield

        def gen_cheb(b):
            xt, at = xt_all[b], at_all[b]
            xn = [xpp.tile([128, F], mdt, tag=f"xn{t}", name=f"xn{t}")
                  for t in range(4)]
            for t in range(4):
                nc.sync.dma_start(xn[t][:], xp_p[b, t * 128:(t + 1) * 128, :])

            zt = [acp.tile([FCH, N], mdt, tag=f"zt{m}", name=f"zt{m}")
                  for m in range(NFCH)]
            for m in range(NFCH):
                psZ = psp.tile([FCH, N], fp, tag="mm", name="psZ")
                for t in range(4):
                    nc.tensor.matmul(psZ[:], xn[t][:, m * FCH:(m + 1) * FCH],
                                     at[t][:], start=(t == 0), stop=(t == 3))
                nc.scalar.copy(zt[m][:], psZ[:])
                yield

            ht = [acp.tile([128, N], mdt, tag=f"ht{k}", name=f"ht{k}")
                  for k in range(len(FH_BLOCKS))]
            for k, (mo, mp_) in enumerate(FH_BLOCKS):
                psH = psp.tile([128, N], fp, tag="mm", name="psH")
                for c in range(NFCH):
                    nc.tensor.matmul(psH[:mp_, :], w1t[0][c][:, mo:mo + mp_],
                                     xt[c][:, :N], start=(c == 0), stop=False)
                for c in range(NFCH):
                    nc.tensor.matmul(psH[:mp_, :], w1t[1][c][:, mo:mo + mp_],
                                     zt[c][:], start=False,
                                     stop=(c == NFCH - 1))
                nc.scalar.activation(ht[k][:mp_, :], psH[:mp_, :], AF.Relu,
                                     bias=b1t[k][:mp_, :], scale=1.0)
                yield

            qt = [acp.tile([128, N], mdt, tag=f"qt{k}", name=f"qt{k}")
                  for k in range(len(FH_BLOCKS))]
            for k, (mo, mp_) in enumerate(FH_BLOCKS):
                psQ = psp.tile([128, N], fp, tag="mm", name="psQ")
                for t in range(4):
                    psT = psp.tile([128, 128], fp, tag="psT", name="psT")
                    nc.tensor.transpose(
                        psT[:, :mp_],
                        ht[k][:mp_, t * 128:(t + 1) * 128].bitcast(fp),
                        ident[:mp_, :mp_])
                    hb = bnd.tile([128, 128], mdt, tag="hb", bufs=3,
                                  name="hb")
                    nc.scalar.copy(hb[:, :mp_], psT[:, :mp_])
                    nc.tensor.matmul(psQ[:mp_, :], hb[:, :mp_], at[t][:],
                                     start=(t == 0), stop=(t == 3))
                nc.scalar.copy(qt[k][:mp_, :], psQ[:mp_, :])
                yield

            for m in range(NFCH):
                psO = psp.tile([FCH, N], fp, tag="mm", name="psO")
                for k, (ko, kp) in enumerate(FH_BLOCKS):
                    nc.tensor.matmul(psO[:],
                                     w2t[0][k][:kp, m * FCH:(m + 1) * FCH],
                                     ht[k][:kp, :], start=(k == 0), stop=False)
                for k, (ko, kp) in enumerate(FH_BLOCKS):
                    nc.tensor.matmul(psO[:],
                                     w2t[1][k][:kp, m * FCH:(m + 1) * FCH],
                                     qt[k][:kp, :], start=False,
                                     stop=(k == len(FH_BLOCKS) - 1))
                ot = otp.tile([FCH, N], fp, tag="ot", name="ot")
                nc.scalar.activation(ot[:], psO[:], AF.Relu, bias=b2t[m][:],
                                     scale=1.0)
                nc.sync.dma_start(out_p[b, m * FCH:(m + 1) * FCH, :], ot[:])
                yield

        # band0 fully; then interleave band1 units with cheb0 units so
        # ChebConv-0 matmuls fill PE slack while DVE paces band1; cheb1 last
        for _ in gen_band(0):
            pass
        g_band1, g_cheb0 = gen_band(1), gen_cheb(0)
        done_b = done_c = False
        while not (done_b and done_c):
            for _ in range(2):
                if not done_b:
                    done_b = next(g_band1, StopIteration) is StopIteration
            if not done_c:
                done_c = next(g_cheb0, StopIteration) is StopIteration
        for _ in gen_cheb(1):
            pass

        if rep_cm is not None:
            rep_cm.__exit__(None, None, None)

    if not nc.is_finalized():
        nc.finalize()
    return nc, WROW


def _prepare(x4, attention):
    """Host prep: flatten, sort by attention, compute band width, pad."""
    X = np.ascontiguousarray(x4.reshape(B, N, F), dtype=np.float32)
    att = np.ascontiguousarray(attention[:, :, 0, 0], dtype=np.float32)
    perms = np.argsort(att, axis=1, kind="stable")
    attp = np.take_along_axis(att, perms, axis=1)
    a64 = attp.astype(np.float64)
    w = 1
    for bi in range(B):
        for d in range(1, N):
            if np.min(a64[bi, d:] - a64[bi, :-d]) <= ATT_THRESH + 1e-6:
                w = max(w, d)
            else:
                break  # windows only widen with d
    w = min(w, N - 1)
    Xp = np.take_along_axis(X, perms[:, :, None], axis=1)
    return Xp, attp, perms, w


def _make_runner(nc):
    """Compile the Bass program into a reusable 8-core sharded jax callable.

    Mirrors concourse.bass2jax.run_bass_via_pjrt's multi-core branch, but
    returns the compiled callable so repeated executions can be timed.
    """
    import jax
    from jax.sharding import Mesh, PartitionSpec
    from jax.experimental.shard_map import shard_map
    from concourse import bass2jax, mybir

    bass2jax.install_neuronx_cc_hook()

    in_names, out_names, out_avals, zero_outs = [], [], [], []
    partition_name = (nc.partition_id_tensor.name
                      if nc.partition_id_tensor else None)
    for alloc in nc.m.functions[0].allocations:
        if not isinstance(alloc, mybir.MemoryLocationSet):
            continue
        name = alloc.memorylocations[0].name
        if alloc.kind == "ExternalInput":
            if name != partition_name:
                in_names.append(name)
        elif alloc.kind == "ExternalOutput":
            shape = tuple(alloc.tensor_shape)
            dtype = mybir.dt.np(alloc.dtype)
            out_names.append(name)
            out_avals.append(jax.core.ShapedArray(shape, dtype))
            zero_outs.append(np.zeros(shape, dtype))
    n_params = len(in_names)
    n_outs = len(out_avals)
    in_names = in_names + out_names
    if partition_name is not None:
        in_names.append(partition_name)
    donate = tuple(range(n_params, n_params + n_outs))

    def _body(*args):
        operands = list(args)
        if partition_name is not None:
            operands.append(bass2jax.partition_id_tensor())
        outs = bass2jax._bass_exec_p.bind(
            *operands,
            out_avals=tuple(out_avals),
            in_names=tuple(in_names),
            out_names=tuple(out_names),
            lowering_input_output_aliases=(),
            sim_require_finite=True,
            sim_require_nnan=True,
            nc=nc,
        )
        return tuple(outs)

    devices = jax.devices()[:NCORES]
    mesh = Mesh(np.asarray(devices), ("core",))
    sharded = jax.jit(
        shard_map(_body, mesh=mesh,
                  in_specs=(PartitionSpec("core"),) * (n_params + n_outs),
                  out_specs=(PartitionSpec("core"),) * n_outs,
                  check_rep=False),
        donate_argnums=donate, keep_unused=True)

    param_order = in_names[:n_params]

    def run(in_maps):
        concat_in = [
            np.concatenate([np.asarray(in_maps[c][nm]) for c in range(NCORES)],
                           axis=0)
            for nm in param_order
        ]
        concat_zeros = [np.zeros((NCORES * z.shape[0], *z.shape[1:]), z.dtype)
                        for z in zero_outs]
        out_arrs = jax.block_until_ready(sharded(*concat_in, *concat_zeros))
        return [
            {nm: np.asarray(out_arrs[i]).reshape(NCORES, *out_avals[i].shape)[c]
             for i, nm in enumerate(out_names)}
            for c in range(NCORES)
        ]

    return {"run": run, "sharded": sharded, "param_order": param_order,
            "zero_outs": zero_outs, "out_names": out_names,
            "out_avals": out_avals, "mesh": mesh}


def _get_runner(w):
    import os
    mm = os.environ.get("KERNEL_MM_DTYPE", "f32r")
    reps = int(os.environ.get("KERNEL_REPS", "1"))
    key = (w, mm, reps)
    if key not in _prog_cache:
        nc, WROW = _build_program(w, mm=mm, reps=reps)
        _prog_cache[key] = (_make_runner(nc), WROW)
    return _prog_cache[key]


def kernel(x4, attention, W1, b1, W2, b2):
    Xp, attp, perms, w = _prepare(x4, attention)
    runner, WROW = _get_runner(w)

    xpt = np.zeros((B, F, WROW), np.float32)
    xpt[:, :, :N] = Xp.transpose(0, 2, 1)
    attp_pad = np.full((B, WROW), 1e9, np.float32)
    attp_pad[:, :N] = attp

    W1 = np.ascontiguousarray(W1, dtype=np.float32)
    W2 = np.ascontiguousarray(W2, dtype=np.float32)
    b1 = np.ascontiguousarray(b1, dtype=np.float32)
    b2 = np.ascontiguousarray(b2, dtype=np.float32)

    c_ones = np.ones((128, 1), np.float32)
    c_onesrow = np.ones((1, N), np.float32)
    c_estep = np.zeros((FCH, 95), np.float32)
    c_estep[:, 47] = 1.0
    c_zeros = np.zeros((128, WROW), np.float32)

    in_maps = []
    for c in range(NCORES):
        sl = slice(c * SPB, (c + 1) * SPB)
        in_maps.append({
            "xp": np.ascontiguousarray(Xp[sl]),
            "xpt": np.ascontiguousarray(xpt[sl]),
            "attp": np.ascontiguousarray(attp_pad[sl]),
            "w1": W1, "b1": b1, "w2": W2, "b2": b2,
            "c_ones": c_ones, "c_onesrow": c_onesrow,
            "c_estep": c_estep, "c_zeros": c_zeros,
        })

    results = runner["run"](in_maps)
    globals()["last_in_maps"] = in_maps
    globals()["last_runner"] = runner

    inv = np.argsort(perms, axis=1)
    out = np.empty((B, N, F), np.float32)
    for c in range(NCORES):
        o = results[c]["outT"]  # [SPB, F, N]
        for s in range(SPB):
            bi = c * SPB + s
            out[bi] = o[s].T[inv[bi]]
    return out



# revision 20
# speedup vs baseline: 261.9691x; 261.9691x over previous
"""Trainium2 Bass kernel: batched ChebConv GNN with L1-distance adjacency.

Pipeline per sample (N=512 nodes, F=625 features, padded to 640):
  1. Sort nodes by attention (host). All pairs with |att_i-att_j| <= 0.05
     then lie within a rank band |i-j| <= w (w computed exactly on host).
  2. Banded pairwise L1 distances on device via the identity
     sum_f |a-b| = 2*sum_f max(a,b) - S_i - S_j, computed with bf16 DVE
     max ops + PE ones-selector matmuls for the feature reduction.
  3. Threshold masks -> banded adjacency rows [2w+1, N] in SBUF. The
     negative offsets are obtained by a skewed DRAM round trip (per-row
     shift = row-granular diagonal AP, contiguous 512-element runs); the
     offset-axis reversal is folded into the PE transpose by using a
     permutation matrix instead of the identity. The transposed band is
     written to a skewed DRAM layout As2[r, (r%128)+k] so that dense
     adjacency tiles [128, 512] load back as plain constant-offset rows.
  4. Degree-normalized ChebConv x2 as bf16 PE matmuls in transposed
     layouts; the 1/deg column scaling is applied in fp32 during the
     PSUM->SBUF copies (DVE multiply against a broadcast dinv tile).
Data parallel over batch: 16 samples, 8 cores, 2 samples/core.
"""

import numpy as np
from contextlib import ExitStack

B, N = 16, 512
F, FP = 625, 640          # features, padded (5 x 128)
FH, FHP = 937, 1024       # hidden features, padded (8 x 128)
NCH, NFH = 5, 8           # 128-chunks of FP / FHP
NCORES = 8
SPB = B // NCORES         # samples per core
DIST_THRESH, ATT_THRESH = 180.0, 0.05
XTW = 560                 # xt row width: N + w + pad
PM = 608                  # msh scratch row pitch (>= 512 + w + 1)
PA = 216                  # As2 row pitch (>= 127 + 2w+1 - 127 ... >= 212)
BD = 4                    # band offsets per DVE instruction

_prog_cache = {}


def _build_program(w, reps=1):
    """Build the SPMD Bass program for band half-width w (<= 63)."""
    import concourse.bass as bass
    import concourse.bacc as bacc
    import concourse.mybir as mybir
    import concourse.tile as tile
    from concourse.masks import make_identity

    assert 1 <= w <= 63, w
    KW = 2 * w + 1

    dt = mybir.dt
    fp = dt.float32
    bf = dt.bfloat16
    AF = mybir.ActivationFunctionType
    OP = mybir.AluOpType
    AP = bass.AP

    nc = bacc.Bacc()
    xpt_p = nc.declare_dram_parameter("xpt", [SPB, FP, XTW], bf, isOutput=False)
    xp_p = nc.declare_dram_parameter("xp", [SPB, N, FP], bf, isOutput=False)
    attp_p = nc.declare_dram_parameter("attp", [SPB, XTW], fp, isOutput=False)
    w1_p = nc.declare_dram_parameter("w1", [2, FP, FHP], bf, isOutput=False)
    b1_p = nc.declare_dram_parameter("b1", [FHP], fp, isOutput=False)
    w2_p = nc.declare_dram_parameter("w2", [2, FHP, FP], bf, isOutput=False)
    b2_p = nc.declare_dram_parameter("b2", [FP], fp, isOutput=False)
    ones_p = nc.declare_dram_parameter("c_ones", [128, 1], bf, isOutput=False)
    onesrow_p = nc.declare_dram_parameter("c_onesrow", [1, N], bf, isOutput=False)
    estep_p = nc.declare_dram_parameter("c_estep", [128, 128], bf, isOutput=False)
    perm_p = nc.declare_dram_parameter("c_perm", [128, 128], bf, isOutput=False)
    out_p = nc.declare_dram_parameter("outT", [SPB, FP, N], fp, isOutput=True)

    # internal DRAM scratch, one set per sample slot
    as2 = [nc.dram_tensor(f"as2_{b}", [4 * 128 * PA], bf) for b in range(SPB)]
    msh = [nc.dram_tensor(f"msh_{b}", [128 * PA], bf) for b in range(SPB)]
    s_scr = [nc.dram_tensor(f"s_scr{b}", [XTW], fp) for b in range(SPB)]
    d_scr = [nc.dram_tensor(f"d_scr{b}", [N], fp) for b in range(SPB)]

    with tile.TileContext(nc) as tc, ExitStack() as ctx:
        cst = ctx.enter_context(tc.tile_pool(name="cst", bufs=1))
        xtp = ctx.enter_context(tc.tile_pool(name="xtp", bufs=1))
        xnp = ctx.enter_context(tc.tile_pool(name="xnp", bufs=1))
        mxp = ctx.enter_context(tc.tile_pool(name="mxp", bufs=2))
        bnd = ctx.enter_context(tc.tile_pool(name="bnd", bufs=1))
        amp = ctx.enter_context(tc.tile_pool(name="amp", bufs=1))
        acp = ctx.enter_context(tc.tile_pool(name="acp", bufs=1))
        wsp = ctx.enter_context(tc.tile_pool(name="wsp", bufs=1))
        otp = ctx.enter_context(tc.tile_pool(name="otp", bufs=2))
        psp = ctx.enter_context(tc.tile_pool(name="psp", bufs=2, space="PSUM"))
        psm = ctx.enter_context(tc.tile_pool(name="psm", bufs=2, space="PSUM"))
        ps1 = ctx.enter_context(tc.tile_pool(name="ps1", bufs=1, space="PSUM"))
        pst = ctx.enter_context(tc.tile_pool(name="pst", bufs=2, space="PSUM"))

        # ---- constants / weights resident in SBUF
        ones = cst.tile([128, 1], bf, tag="ones")
        nc.sync.dma_start(ones[:], ones_p[:, :])
        estep = cst.tile([128, 128], bf, tag="estep")
        nc.sync.dma_start(estep[:], estep_p[:, :])
        perm = cst.tile([128, 128], bf, tag="perm")
        nc.sync.dma_start(perm[:], perm_p[:, :])
        ident = cst.tile([128, 128], bf, tag="ident")
        make_identity(nc, ident[:])
        zsrc = cst.tile([128, PA], bf, tag="zsrc")
        nc.gpsimd.memset(zsrc[:], 0.0)

        w1t = [[wsp.tile([128, FHP], bf, tag=f"w1t{k}{c}", name=f"w1t{k}{c}")
                for c in range(NCH)] for k in range(2)]
        for k in range(2):
            for c in range(NCH):
                nc.sync.dma_start(w1t[k][c][:],
                                  w1_p[k, c * 128:(c + 1) * 128, :])
        w2t = [[wsp.tile([128, FP], bf, tag=f"w2t{k}{j}", name=f"w2t{k}{j}")
                for j in range(NFH)] for k in range(2)]
        for k in range(2):
            for j in range(NFH):
                nc.sync.dma_start(w2t[k][j][:],
                                  w2_p[k, j * 128:(j + 1) * 128, :])
        b1t = [wsp.tile([128, 1], fp, tag=f"b1t{j}", name=f"b1t{j}")
               for j in range(NFH)]
        for j in range(NFH):
            nc.sync.dma_start(b1t[j][:], b1_p[j * 128:(j + 1) * 128])
        b2t = [wsp.tile([128, 1], fp, tag=f"b2t{m}", name=f"b2t{m}")
               for m in range(NCH)]
        for m in range(NCH):
            nc.sync.dma_start(b2t[m][:], b2_p[m * 128:(m + 1) * 128])

        # per-sample persistent tiles (zeroed once; geometry-stable reuse)
        combF = [cst.tile([128, 512], bf, tag=f"combF{b}", name=f"combF{b}")
                 for b in range(SPB)]
        at_t = [[amp.tile([128, 512], bf, tag=f"at{b}{t}", name=f"at{b}{t}")
                 for t in range(4)] for b in range(SPB)]
        dinvB = [bnd.tile([128, 512], fp, tag=f"dinvB{b}", name=f"dinvB{b}")
                 for b in range(SPB)]
        for b in range(SPB):
            nc.gpsimd.memset(combF[b][:], 0.0)
            # diagonal (identity) row of the band lives at partition 63;
            # engine ops can't target unaligned partition bases, DMA can.
            nc.sync.dma_start(combF[b][63:64, :N], onesrow_p[:, :])
            for t in range(4):
                nc.gpsimd.memset(at_t[b][t][:], 0.0)
            # zero-fill DRAM scratch (guard zeros; geometry-stable after)
            for t in range(4):
                nc.sync.dma_start(AP(as2[b], t * 128 * PA, [[1, 128 * PA]]),
                                  zsrc[:])
            nc.sync.dma_start(AP(msh[b], 0, [[1, 128 * PA]]), zsrc[:])
            nc.sync.dma_start(AP(s_scr[b], N, [[1, XTW - N]]),
                              zsrc[0:1, 0:2 * (XTW - N)].bitcast(fp))

        # PE warmup: ~16 x 216-cycle matmuls on zeros keeps HAM busy while
        # the input DMAs land, so the band matmuls start at full clock.
        psW = psm.tile([w, N], fp, tag="psM", name="psW")
        for i in range(16):
            nc.tensor.matmul(psW[:, :216], ident[:, :w], zsrc[:, :216],
                             start=(i == 0), stop=(i == 15))

        rep_cm = tc.For_i(0, reps, 1) if reps > 1 else None
        if rep_cm is not None:
            rep_cm.__enter__()

        xt_all = [None] * SPB

        def gen_band(b):
            xt = [xtp.tile([128, XTW], bf, tag=f"xt{b}{c}", name=f"xt{b}{c}")
                  for c in range(NCH)]
            for c in range(NCH):
                nc.sync.dma_start(xt[c][:], xpt_p[b, c * 128:(c + 1) * 128, :])
            xt_all[b] = xt
            # S row sums via ones matmul (fp32 accumulate)
            psS = ps1.tile([1, N], fp, tag="ps1", name="psS")
            for c in range(NCH):
                nc.tensor.matmul(psS[:], ones[:], xt[c][:, :N],
                                 start=(c == 0), stop=(c == NCH - 1))
            srow = bnd.tile([1, N], fp, tag="srow", name="srow")
            nc.scalar.copy(srow[:], psS[:])
            nc.sync.dma_start(AP(s_scr[b], 0, [[1, N]]), srow[:])

            # banded max sums: psM[di, i] = sum_f max(x[i,f], x[i+di+1,f])
            psM = psm.tile([w, N], fp, tag="psM", name="psM")
            nbat = (w + BD - 1) // BD
            for bi in range(nbat):
                db0 = bi * BD
                nb = min(BD, w - db0)
                for c in range(NCH):
                    mxb = mxp.tile([128, BD * N], bf, tag="mx", name="mxb")
                    base = xt[c][:, 0:N]
                    in0 = bass.AP(base.tensor, base.offset,
                                  [list(base.ap[0]), [0, nb], [1, N]])
                    in1 = bass.AP(base.tensor, base.offset + 1 + db0,
                                  [list(base.ap[0]), [1, nb], [1, N]])
                    nc.vector.tensor_tensor(
                        out=mxb[:, :nb * N], in0=in0, in1=in1, op=OP.max)
                    for j in range(nb):
                        di = db0 + j
                        nc.tensor.matmul(
                            psM[:, :], estep[:, 63 - di:63 - di + w],
                            mxb[:, j * N:(j + 1) * N],
                            start=(di == 0 and c == 0),
                            stop=(di == w - 1 and c == NCH - 1))
                    yield

            # thresholds -> positive-offset masks, written into combF rows
            # [w+1, 2w+1) (ascending d); then skewed round trip builds the
            # shifted masks in rows [0, w).
            sshift = bnd.tile([w, N], fp, tag="sshift", name="sshift")
            nc.sync.dma_start(sshift[:], AP(s_scr[b], 1, [[1, w], [1, N]]))
            sb_t = bnd.tile([w, N], fp, tag="sb", name="sb_t")
            nc.sync.dma_start(sb_t[:], AP(s_scr[b], 0, [[0, w], [1, N]]))
            ashift = bnd.tile([w, N], fp, tag="ashift", name="ashift")
            nc.sync.dma_start(ashift[:],
                              AP(attp_p, b * XTW + 1, [[1, w], [1, N]]))
            ab_t = bnd.tile([w, N], fp, tag="ab", name="ab_t")
            nc.sync.dma_start(ab_t[:], AP(attp_p, b * XTW, [[0, w], [1, N]]))
            nc.vector.scalar_tensor_tensor(
                out=sb_t[:], in0=sb_t[:], scalar=DIST_THRESH, in1=sshift[:],
                op0=OP.add, op1=OP.add)
            nc.vector.scalar_tensor_tensor(
                out=sshift[:], in0=psM[:], scalar=2.0, in1=sb_t[:],
                op0=OP.mult, op1=OP.is_le)
            nc.vector.tensor_sub(ashift[:], ashift[:], ab_t[:])
            nc.vector.tensor_scalar(ab_t[:], ashift[:], ATT_THRESH, None,
                                    op0=OP.is_le)
            nc.vector.tensor_mul(combF[b][0:w, :N],
                                 sshift[:], ab_t[:])
            yield
            # skewed write: msh[p, (p+1)+x] = mask_{p+1}[x]
            nc.sync.dma_start(AP(msh[b], 1, [[PM + 1, w], [1, N]]),
                              combF[b][0:w, :N])
            # plain read: combF[64+q, i] = msh[q, i] = mask_{q+1}[i-(q+1)]
            nc.sync.dma_start(combF[b][64:64 + w, :N],
                              AP(msh[b], 0, [[PM, w], [1, N]]))
            # degree from all band rows + diagonal ones row
            psD = ps1.tile([1, N], fp, tag="ps1", name="psD")
            nc.tensor.matmul(psD[:], ones[:], combF[b][:, :N],
                             start=True, stop=True)
            drow = bnd.tile([1, N], fp, tag="drow", name="drow")
            nc.vector.reciprocal(drow[:], psD[:])
            nc.sync.dma_start(AP(d_scr[b], 0, [[1, N]]), drow[:])
            nc.sync.dma_start(dinvB[b][:], AP(d_scr[b], 0, [[0, 128], [1, N]]))
            yield
            # permuted transpose -> skewed As2 write -> dense at tiles
            for t in range(4):
                psT = pst.tile([128, 128], bf, tag="psT", name="psT")
                nc.tensor.transpose(psT[:],
                                    combF[b][:, t * 128:(t + 1) * 128],
                                    perm[:])
                ct = bnd.tile([128, KW], bf, tag="ct", bufs=2, name="ct")
                nc.scalar.copy(ct[:], psT[:, :KW])
                nc.sync.dma_start(
                    AP(as2[b], t * 128 * PA, [[PA + 1, 128], [1, KW]]),
                    ct[:])
                yield
            for t in range(4):
                o0 = t * 128 - w
                c0 = max(0, o0)
                c1 = min(N, o0 + KW + 127)
                m0 = c0 - o0
                nc.sync.dma_start(
                    at_t[b][t][:, c0:c1],
                    AP(as2[b], t * 128 * PA + m0, [[PA, 128], [1, c1 - c0]]))
            yield

        def gen_cheb(b):
            xt, at = xt_all[b], at_t[b]
            xn = [xnp.tile([128, FP], bf, tag=f"xn{b}{t}", name=f"xn{b}{t}")
                  for t in range(4)]
            for t in range(4):
                nc.sync.dma_start(xn[t][:], xp_p[b, t * 128:(t + 1) * 128, :])

            zt = [acp.tile([128, N], bf, tag=f"zt{m}", name=f"zt{m}")
                  for m in range(NCH)]
            for m in range(NCH):
                psZ = psp.tile([128, N], fp, tag="mm", name="psZ")
                for t in range(4):
                    nc.tensor.matmul(psZ[:], xn[t][:, m * 128:(m + 1) * 128],
                                     at[t][:], start=(t == 0), stop=(t == 3))
                nc.vector.tensor_mul(zt[m][:], psZ[:], dinvB[b][:])
                yield

            ht = [acp.tile([128, N], bf, tag=f"ht{j}", name=f"ht{j}")
                  for j in range(NFH)]
            for j in range(NFH):
                psH = psp.tile([128, N], fp, tag="mm", name="psH")
                for c in range(NCH):
                    nc.tensor.matmul(psH[:], w1t[0][c][:, j * 128:(j + 1) * 128],
                                     xt[c][:, :N], start=(c == 0), stop=False)
                for c in range(NCH):
                    nc.tensor.matmul(psH[:], w1t[1][c][:, j * 128:(j + 1) * 128],
                                     zt[c][:], start=False, stop=(c == NCH - 1))
                nc.scalar.activation(ht[j][:], psH[:], AF.Relu,
                                     bias=b1t[j][:], scale=1.0)
                yield

            qt = [acp.tile([128, N], bf, tag=f"qt{j}", name=f"qt{j}")
                  for j in range(NFH)]
            for j in range(NFH):
                psQ = psp.tile([128, N], fp, tag="mm", name="psQ")
                for t in range(4):
                    psT2 = pst.tile([128, 128], bf, tag="psT", name="psT2")
                    nc.tensor.transpose(psT2[:],
                                        ht[j][:, t * 128:(t + 1) * 128],
                                        ident[:])
                    hb = bnd.tile([128, 128], bf, tag="hb", bufs=3, name="hb")
                    nc.scalar.copy(hb[:], psT2[:])
                    nc.tensor.matmul(psQ[:], hb[:], at[t][:],
                                     start=(t == 0), stop=(t == 3))
                nc.vector.tensor_mul(qt[j][:], psQ[:], dinvB[b][:])
                yield

            for m in range(NCH):
                psO = psp.tile([128, N], fp, tag="mm", name="psO")
                for j in range(NFH):
                    nc.tensor.matmul(psO[:], w2t[0][j][:, m * 128:(m + 1) * 128],
                                     ht[j][:], start=(j == 0), stop=False)
                for j in range(NFH):
                    nc.tensor.matmul(psO[:], w2t[1][j][:, m * 128:(m + 1) * 128],
                                     qt[j][:], start=False, stop=(j == NFH - 1))
                ot = otp.tile([128, N], fp, tag="ot", name="ot")
                nc.scalar.activation(ot[:], psO[:], AF.Relu,
                                     bias=b2t[m][:], scale=1.0)
                nc.sync.dma_start(out_p[b, m * 128:(m + 1) * 128, :], ot[:])
                yield

        # band0 fully; then interleave band1 with cheb0 so ChebConv-0
        # matmuls fill PE slack while DVE paces band1; cheb1 last.
        for _ in gen_band(0):
            pass
        g_band1, g_cheb0 = gen_band(1), gen_cheb(0)
        done_b = done_c = False
        while not (done_b and done_c):
            for _ in range(2):
                if not done_b:
                    done_b = next(g_band1, StopIteration) is StopIteration
            if not done_c:
                done_c = next(g_cheb0, StopIteration) is StopIteration
        for _ in gen_cheb(1):
            pass

        if rep_cm is not None:
            rep_cm.__exit__(None, None, None)

    if not nc.is_finalized():
        nc.finalize()
    return nc


def _prepare(x4, attention):
    """Host prep: flatten, sort by attention, compute band width."""
    X = np.ascontiguousarray(x4.reshape(B, N, F), dtype=np.float32)
    att = np.ascontiguousarray(attention[:, :, 0, 0], dtype=np.float32)
    perms = np.argsort(att, axis=1, kind="stable")
    attp = np.take_along_axis(att, perms, axis=1)
    a64 = attp.astype(np.float64)
    w = 1
    for bi in range(B):
        for d in range(1, N):
            if np.min(a64[bi, d:] - a64[bi, :-d]) <= ATT_THRESH + 1e-6:
                w = max(w, d)
            else:
                break  # windows only widen with d
    w = min(w, N - 1)
    Xp = np.take_along_axis(X, perms[:, :, None], axis=1)
    return Xp, attp, perms, w


def _make_runner(nc):
    """Compile the Bass program into a reusable 8-core sharded jax callable."""
    import jax
    from jax.sharding import Mesh, PartitionSpec
    from jax.experimental.shard_map import shard_map
    from concourse import bass2jax, mybir

    bass2jax.install_neuronx_cc_hook()

    in_names, out_names, out_avals, zero_outs = [], [], [], []
    partition_name = (nc.partition_id_tensor.name
                      if nc.partition_id_tensor else None)
    for alloc in nc.m.functions[0].allocations:
        if not isinstance(alloc, mybir.MemoryLocationSet):
            continue
        name = alloc.memorylocations[0].name
        if alloc.kind == "ExternalInput":
            if name != partition_name:
                in_names.append(name)
        elif alloc.kind == "ExternalOutput":
            shape = tuple(alloc.tensor_shape)
            dtype = mybir.dt.np(alloc.dtype)
            out_names.append(name)
            out_avals.append(jax.core.ShapedArray(shape, dtype))
            zero_outs.append(np.zeros(shape, dtype))
    n_params = len(in_names)
    n_outs = len(out_avals)
    in_names = in_names + out_names
    if partition_name is not None:
        in_names.append(partition_name)
    donate = tuple(range(n_params, n_params + n_outs))

    def _body(*args):
        operands = list(args)
        if partition_name is not None:
            operands.append(bass2jax.partition_id_tensor())
        outs = bass2jax._bass_exec_p.bind(
            *operands,
            out_avals=tuple(out_avals),
            in_names=tuple(in_names),
            out_names=tuple(out_names),
            lowering_input_output_aliases=(),
            sim_require_finite=True,
            sim_require_nnan=True,
            nc=nc,
        )
        return tuple(outs)

    devices = jax.devices()[:NCORES]
    mesh = Mesh(np.asarray(devices), ("core",))
    sharded = jax.jit(
        shard_map(_body, mesh=mesh,
                  in_specs=(PartitionSpec("core"),) * (n_params + n_outs),
                  out_specs=(PartitionSpec("core"),) * n_outs,
                  check_rep=False),
        donate_argnums=donate, keep_unused=True)

    param_order = in_names[:n_params]

    def run(in_maps):
        concat_in = [
            np.concatenate([np.asarray(in_maps[c][nm]) for c in range(NCORES)],
                           axis=0)
            for nm in param_order
        ]
        concat_zeros = [np.zeros((NCORES * z.shape[0], *z.shape[1:]), z.dtype)
                        for z in zero_outs]
        out_arrs = jax.block_until_ready(sharded(*concat_in, *concat_zeros))
        return [
            {nm: np.asarray(out_arrs[i]).reshape(NCORES, *out_avals[i].shape)[c]
             for i, nm in enumerate(out_names)}
            for c in range(NCORES)
        ]

    return {"run": run, "sharded": sharded, "param_order": param_order,
            "zero_outs": zero_outs, "out_names": out_names,
            "out_avals": out_avals, "mesh": mesh}


def _get_runner(w):
    import os
    reps = int(os.environ.get("KERNEL_REPS", "1"))
    key = (w, reps)
    if key not in _prog_cache:
        nc = _build_program(w, reps=reps)
        _prog_cache[key] = (_make_runner(nc), nc)
    return _prog_cache[key]


def kernel(x4, attention, W1, b1, W2, b2):
    from concourse import mybir
    bfnp = mybir.dt.np(mybir.dt.bfloat16)

    Xp, attp, perms, w = _prepare(x4, attention)
    runner, nc = _get_runner(w)

    xpt = np.zeros((B, FP, XTW), np.float32)
    xpt[:, :F, :N] = Xp.transpose(0, 2, 1)
    xpt = xpt.astype(bfnp)
    xp = np.zeros((B, N, FP), np.float32)
    xp[:, :, :F] = Xp
    xp = xp.astype(bfnp)
    attp_pad = np.full((B, XTW), 1e9, np.float32)
    attp_pad[:, :N] = attp

    w1 = np.zeros((2, FP, FHP), np.float32)
    w1[:, :F, :FH] = np.asarray(W1, np.float32)
    w1 = w1.astype(bfnp)
    w2 = np.zeros((2, FHP, FP), np.float32)
    w2[:, :FH, :F] = np.asarray(W2, np.float32)
    w2 = w2.astype(bfnp)
    b1p = np.zeros((FHP,), np.float32)
    b1p[:FH] = np.asarray(b1, np.float32)
    b2p = np.zeros((FP,), np.float32)
    b2p[:F] = np.asarray(b2, np.float32)

    c_ones = np.ones((128, 1), np.float32).astype(bfnp)
    c_onesrow = np.ones((1, N), np.float32).astype(bfnp)
    c_estep = np.zeros((128, 128), np.float32)
    c_estep[:, 63] = 1.0
    c_estep = c_estep.astype(bfnp)
    # transpose permutation: combF row q<w holds mask d=q+1 -> band col
    # w+d; row 63 diagonal -> col w; row 64+q holds shifted mask d=q+1
    # -> band col w-1-q; all other rows map to nothing (zero).
    c_perm = np.zeros((128, 128), np.float32)
    for q in range(w):
        c_perm[q, w + q + 1] = 1.0
        c_perm[64 + q, w - 1 - q] = 1.0
    c_perm[63, w] = 1.0
    c_perm = c_perm.astype(bfnp)

    in_maps = []
    for c in range(NCORES):
        sl = slice(c * SPB, (c + 1) * SPB)
        in_maps.append({
            "xpt": np.ascontiguousarray(xpt[sl]),
            "xp": np.ascontiguousarray(xp[sl]),
            "attp": np.ascontiguousarray(attp_pad[sl]),
            "w1": w1, "b1": b1p, "w2": w2, "b2": b2p,
            "c_ones": c_ones, "c_onesrow": c_onesrow,
            "c_estep": c_estep, "c_perm": c_perm,
        })

    results = runner["run"](in_maps)
    globals()["last_in_maps"] = in_maps
    globals()["last_runner"] = runner
    globals()["last_nc"] = nc

    inv = np.argsort(perms, axis=1)
    out = np.empty((B, N, F), np.float32)
    for c in range(NCORES):
        o = results[c]["outT"]  # [SPB, FP, N]
        for s in range(SPB):
            bi = c * SPB + s
            out[bi] = o[s, :F, :].T[inv[bi]]
    return out


# revision 34
# speedup vs baseline: 301.4113x; 1.1506x over previous
"""Trainium2 Bass kernel: batched ChebConv GNN with L1-distance adjacency.

Pipeline per sample (N=512 nodes, F=625 features, padded to 640):
  1. Sort nodes by attention (host). All pairs with |att_i-att_j| <= 0.05
     then lie within a rank band |i-j| <= w (w computed exactly on host).
  2. Banded pairwise L1 distances on device via the identity
     sum_f |a-b| = 2*sum_f max(a,b) - S_i - S_j, computed with bf16 DVE
     max ops + PE ones-selector matmuls for the feature reduction.
  3. Threshold masks -> banded adjacency rows [2w+1, N] in SBUF. The
     negative offsets are obtained by a skewed DRAM round trip (per-row
     shift = row-granular diagonal AP, contiguous 512-element runs); the
     offset-axis reversal is folded into the PE transpose by using a
     permutation matrix instead of the identity. The transposed band is
     written to a skewed DRAM layout As2[r, (r%128)+k] so that dense
     adjacency tiles [128, 512] load back as plain constant-offset rows.
  4. Degree-normalized ChebConv x2 as bf16 PE matmuls in transposed
     layouts; the 1/deg column scaling is applied in fp32 during the
     PSUM->SBUF copies (DVE multiply against a broadcast dinv tile).
Data parallel over batch: 16 samples, 8 cores, 2 samples/core.
"""

import numpy as np
from contextlib import ExitStack

B, N = 16, 512
F, FP = 625, 640          # features, padded (5 x 128)
FH, FHP = 937, 1024       # hidden features, padded (8 x 128)
NCH, NFH = 5, 8           # 128-chunks of FP / FHP
NCORES = 8
SPB = B // NCORES         # samples per core
DIST_THRESH, ATT_THRESH = 180.0, 0.05
XTW = 560                 # xt row width: N + w + pad
PM = 608                  # msh scratch row pitch (>= 512 + w + 1)
PA = 216                  # As2 row pitch (>= 127 + 2w+1 - 127 ... >= 212)
BD = 4                    # band offsets per DVE instruction

_prog_cache = {}


def _build_program(w, reps=1):
    """Build the SPMD Bass program for band half-width w (<= 63)."""
    import concourse.bass as bass
    import concourse.bacc as bacc
    import concourse.mybir as mybir
    import concourse.tile as tile
    from concourse.masks import make_identity

    assert 1 <= w <= 63, w
    KW = 2 * w + 1

    dt = mybir.dt
    fp = dt.float32
    bf = dt.bfloat16
    AF = mybir.ActivationFunctionType
    OP = mybir.AluOpType
    AP = bass.AP

    nc = bacc.Bacc()
    xpt_p = nc.declare_dram_parameter("xpt", [SPB, FP, XTW], bf, isOutput=False)
    xp_p = nc.declare_dram_parameter("xp", [SPB, N, FP], bf, isOutput=False)
    attp_p = nc.declare_dram_parameter("attp", [SPB, XTW], fp, isOutput=False)
    w1_p = nc.declare_dram_parameter("w1", [2, FP, FHP], bf, isOutput=False)
    b1_p = nc.declare_dram_parameter("b1", [FHP], fp, isOutput=False)
    w2_p = nc.declare_dram_parameter("w2", [2, FHP, FP], bf, isOutput=False)
    b2_p = nc.declare_dram_parameter("b2", [FP], fp, isOutput=False)
    ones_p = nc.declare_dram_parameter("c_ones", [128, 1], bf, isOutput=False)
    onesrow_p = nc.declare_dram_parameter("c_onesrow", [1, N], bf, isOutput=False)
    estep_p = nc.declare_dram_parameter("c_estep", [128, 128], bf, isOutput=False)
    perm_p = nc.declare_dram_parameter("c_perm", [128, 128], bf, isOutput=False)
    out_p = nc.declare_dram_parameter("outT", [SPB, FP, N], fp, isOutput=True)

    # internal DRAM scratch, one set per sample slot
    as2 = [nc.dram_tensor(f"as2_{b}", [4 * 128 * PA], bf) for b in range(SPB)]
    msh = [nc.dram_tensor(f"msh_{b}", [128 * PA], bf) for b in range(SPB)]
    s_scr = [nc.dram_tensor(f"s_scr{b}", [XTW], fp) for b in range(SPB)]
    d_scr = [nc.dram_tensor(f"d_scr{b}", [N], bf) for b in range(SPB)]

    with tile.TileContext(nc) as tc, ExitStack() as ctx:
        cst = ctx.enter_context(tc.tile_pool(name="cst", bufs=1))
        xtp = ctx.enter_context(tc.tile_pool(name="xtp", bufs=1))
        xnp = ctx.enter_context(tc.tile_pool(name="xnp", bufs=1))
        mxp = ctx.enter_context(tc.tile_pool(name="mxp", bufs=2))
        bnd = ctx.enter_context(tc.tile_pool(name="bnd", bufs=1))
        amp = ctx.enter_context(tc.tile_pool(name="amp", bufs=1))
        acp = ctx.enter_context(tc.tile_pool(name="acp", bufs=1))
        wsp = ctx.enter_context(tc.tile_pool(name="wsp", bufs=1))
        otp = ctx.enter_context(tc.tile_pool(name="otp", bufs=2))
        psp = ctx.enter_context(tc.tile_pool(name="psp", bufs=2, space="PSUM"))
        psm = ctx.enter_context(tc.tile_pool(name="psm", bufs=2, space="PSUM"))
        ps1 = ctx.enter_context(tc.tile_pool(name="ps1", bufs=1, space="PSUM"))
        pst = ctx.enter_context(tc.tile_pool(name="pst", bufs=2, space="PSUM"))

        # ---- constants / weights resident in SBUF
        ones = cst.tile([128, 1], bf, tag="ones")
        nc.scalar.dma_start(ones[:], ones_p[:, :])
        estep = cst.tile([128, 128], bf, tag="estep")
        nc.scalar.dma_start(estep[:], estep_p[:, :])
        perm = cst.tile([128, 128], bf, tag="perm")
        nc.scalar.dma_start(perm[:], perm_p[:, :])
        ident = cst.tile([128, 128], bf, tag="ident")
        make_identity(nc, ident[:])
        zsrc = cst.tile([128, PA], bf, tag="zsrc")
        nc.gpsimd.memset(zsrc[:], 0.0)

        # weights go on the Activation HWDGE queue so they don't delay the
        # sample-0 input loads issued on the Sync queue by gen_band(0).
        w1t = [[wsp.tile([128, FHP], bf, tag=f"w1t{k}{c}", name=f"w1t{k}{c}")
                for c in range(NCH)] for k in range(2)]
        for k in range(2):
            for c in range(NCH):
                nc.scalar.dma_start(w1t[k][c][:],
                                    w1_p[k, c * 128:(c + 1) * 128, :])
        w2t = [[wsp.tile([128, FP], bf, tag=f"w2t{k}{j}", name=f"w2t{k}{j}")
                for j in range(NFH)] for k in range(2)]
        for k in range(2):
            for j in range(NFH):
                nc.scalar.dma_start(w2t[k][j][:],
                                    w2_p[k, j * 128:(j + 1) * 128, :])
        b1t = [wsp.tile([128, 1], fp, tag=f"b1t{j}", name=f"b1t{j}")
               for j in range(NFH)]
        for j in range(NFH):
            nc.scalar.dma_start(b1t[j][:], b1_p[j * 128:(j + 1) * 128])
        b2t = [wsp.tile([128, 1], fp, tag=f"b2t{m}", name=f"b2t{m}")
               for m in range(NCH)]
        for m in range(NCH):
            nc.scalar.dma_start(b2t[m][:], b2_p[m * 128:(m + 1) * 128])

        # per-sample persistent tiles (zeroed once; geometry-stable reuse)
        combF = [cst.tile([128, 512], bf, tag=f"combF{b}", name=f"combF{b}")
                 for b in range(SPB)]
        at_t = [[amp.tile([128, 512], bf, tag=f"at{b}{t}", name=f"at{b}{t}")
                 for t in range(4)] for b in range(SPB)]
        dinvB = [bnd.tile([128, 512], bf, tag=f"dinvB{b}", name=f"dinvB{b}")
                 for b in range(SPB)]
        for b in range(SPB):
            nc.gpsimd.memset(combF[b][:], 0.0)
            # diagonal (identity) row of the band lives at partition 63;
            # engine ops can't target unaligned partition bases, DMA can.
            nc.scalar.dma_start(combF[b][63:64, :N], onesrow_p[:, :])
            for t in range(4):
                nc.gpsimd.memset(at_t[b][t][:], 0.0)
            # zero-fill DRAM scratch (guard zeros; geometry-stable after)
            for t in range(4):
                nc.scalar.dma_start(AP(as2[b], t * 128 * PA, [[1, 128 * PA]]),
                                    zsrc[:])
            nc.scalar.dma_start(AP(msh[b], 0, [[1, 128 * PA]]), zsrc[:])
            nc.scalar.dma_start(AP(s_scr[b], N, [[1, XTW - N]]),
                                zsrc[0:1, 0:2 * (XTW - N)].bitcast(fp))

        # PE warmup: ~16 x 216-cycle matmuls on zeros keeps HAM busy while
        # the input DMAs land, so the band matmuls start at full clock.
        psW = psm.tile([w, N], fp, tag="psM", name="psW")
        for i in range(16):
            nc.tensor.matmul(psW[:, :216], ident[:, :w], zsrc[:, :216],
                             start=(i == 0), stop=(i == 15))

        rep_cm = tc.For_i(0, reps, 1) if reps > 1 else None
        if rep_cm is not None:
            rep_cm.__enter__()

        xt_all = [None] * SPB

        def gen_band(b):
            xt = [xtp.tile([128, XTW], bf, tag=f"xt{b}{c}", name=f"xt{b}{c}")
                  for c in range(NCH)]
            for c in range(NCH):
                nc.sync.dma_start(xt[c][:], xpt_p[b, c * 128:(c + 1) * 128, :])
            xt_all[b] = xt
            # S row sums via ones matmul (fp32 accumulate)
            psS = ps1.tile([1, N], fp, tag="ps1", name="psS")
            for c in range(NCH):
                nc.tensor.matmul(psS[:], ones[:], xt[c][:, :N],
                                 start=(c == 0), stop=(c == NCH - 1))
            srow = bnd.tile([1, N], fp, tag="srow", name="srow")
            nc.scalar.copy(srow[:], psS[:])
            nc.sync.dma_start(AP(s_scr[b], 0, [[1, N]]), srow[:])

            # banded max sums: psM[di, i] = sum_f max(x[i,f], x[i+di+1,f])
            psM = psm.tile([w, N], fp, tag="psM", name="psM")
            nbat = (w + BD - 1) // BD
            for bi in range(nbat):
                db0 = bi * BD
                nb = min(BD, w - db0)
                mxb = [mxp.tile([128, BD * N], bf, tag=f"mx{c}",
                                name=f"mxb{c}") for c in range(NCH)]
                for c in range(NCH):
                    base = xt[c][:, 0:N]
                    in0 = bass.AP(base.tensor, base.offset,
                                  [list(base.ap[0]), [0, nb], [1, N]])
                    in1 = bass.AP(base.tensor, base.offset + 1 + db0,
                                  [list(base.ap[0]), [1, nb], [1, N]])
                    nc.vector.tensor_tensor(
                        out=mxb[c][:, :nb * N], in0=in0, in1=in1, op=OP.max)
                # j-outer / c-inner: 5 consecutive matmuls share one
                # estep slice (one weight load with LDW dedup)
                for j in range(nb):
                    di = db0 + j
                    for c in range(NCH):
                        nc.tensor.matmul(
                            psM[:, :], estep[:, 63 - di:63 - di + w],
                            mxb[c][:, j * N:(j + 1) * N],
                            start=(di == 0 and c == 0),
                            stop=(di == w - 1 and c == NCH - 1))
                    yield

            # thresholds -> positive-offset masks, written into combF rows
            # [w+1, 2w+1) (ascending d); then skewed round trip builds the
            # shifted masks in rows [0, w).
            sshift = bnd.tile([w, N], fp, tag="sshift", name="sshift")
            nc.sync.dma_start(sshift[:], AP(s_scr[b], 1, [[1, w], [1, N]]))
            sb_t = bnd.tile([w, N], fp, tag="sb", name="sb_t")
            nc.sync.dma_start(sb_t[:], AP(s_scr[b], 0, [[0, w], [1, N]]))
            ashift = bnd.tile([w, N], fp, tag="ashift", name="ashift")
            nc.sync.dma_start(ashift[:],
                              AP(attp_p, b * XTW + 1, [[1, w], [1, N]]))
            ab_t = bnd.tile([w, N], fp, tag="ab", name="ab_t")
            nc.sync.dma_start(ab_t[:], AP(attp_p, b * XTW, [[0, w], [1, N]]))
            nc.vector.scalar_tensor_tensor(
                out=sb_t[:], in0=sb_t[:], scalar=DIST_THRESH, in1=sshift[:],
                op0=OP.add, op1=OP.add)
            nc.vector.scalar_tensor_tensor(
                out=sshift[:], in0=psM[:], scalar=2.0, in1=sb_t[:],
                op0=OP.mult, op1=OP.is_le)
            nc.vector.tensor_sub(ashift[:], ashift[:], ab_t[:])
            nc.vector.tensor_scalar(ab_t[:], ashift[:], ATT_THRESH, None,
                                    op0=OP.is_le)
            nc.vector.tensor_mul(combF[b][0:w, :N],
                                 sshift[:], ab_t[:])
            yield
            # skewed write: msh[p, (p+1)+x] = mask_{p+1}[x]
            nc.sync.dma_start(AP(msh[b], 1, [[PM + 1, w], [1, N]]),
                              combF[b][0:w, :N])
            # plain read: combF[64+q, i] = msh[q, i] = mask_{q+1}[i-(q+1)]
            nc.sync.dma_start(combF[b][64:64 + w, :N],
                              AP(msh[b], 0, [[PM, w], [1, N]]))
            # degree from all band rows + diagonal ones row
            psD = ps1.tile([1, N], fp, tag="ps1", name="psD")
            nc.tensor.matmul(psD[:], ones[:], combF[b][:, :N],
                             start=True, stop=True)
            drow = bnd.tile([1, N], bf, tag="drow", name="drow")
            with nc.allow_low_precision(reason="bf16 1/deg column scale"):
                nc.vector.reciprocal(drow[:], psD[:])
            nc.sync.dma_start(AP(d_scr[b], 0, [[1, N]]), drow[:])
            nc.sync.dma_start(dinvB[b][:], AP(d_scr[b], 0, [[0, 128], [1, N]]))
            yield
            # permuted transpose -> skewed As2 write -> dense at tiles
            for t in range(4):
                psT = pst.tile([128, 128], bf, tag="psT", name="psT")
                nc.tensor.transpose(psT[:],
                                    combF[b][:, t * 128:(t + 1) * 128],
                                    perm[:])
                ct = bnd.tile([128, KW], bf, tag="ct", bufs=2, name="ct")
                nc.scalar.copy(ct[:], psT[:, :KW])
                nc.sync.dma_start(
                    AP(as2[b], t * 128 * PA, [[PA + 1, 128], [1, KW]]),
                    ct[:])
                yield
            for t in range(4):
                o0 = t * 128 - w
                c0 = max(0, o0)
                c1 = min(N, o0 + KW + 127)
                m0 = c0 - o0
                nc.sync.dma_start(
                    at_t[b][t][:, c0:c1],
                    AP(as2[b], t * 128 * PA + m0, [[PA, 128], [1, c1 - c0]]))
                # fold the 1/deg column scaling into the adjacency tiles
                # (bf16 dinv; zt/qt copies then run on the idle ACT engine)
                nc.vector.tensor_mul(at_t[b][t][:, c0:c1],
                                     at_t[b][t][:, c0:c1],
                                     dinvB[b][:, c0:c1])
            yield

        def gen_cheb(b):
            xt, at = xt_all[b], at_t[b]
            xn = [xnp.tile([128, FP], bf, tag=f"xn{b}{t}", name=f"xn{b}{t}")
                  for t in range(4)]
            for t in range(4):
                nc.sync.dma_start(xn[t][:], xp_p[b, t * 128:(t + 1) * 128, :])

            zt = [acp.tile([128, N], bf, tag=f"zt{m}", name=f"zt{m}")
                  for m in range(NCH)]
            for m in range(NCH):
                psZ = psp.tile([128, N], fp, tag="mm", name="psZ")
                for t in range(4):
                    nc.tensor.matmul(psZ[:], xn[t][:, m * 128:(m + 1) * 128],
                                     at[t][:], start=(t == 0), stop=(t == 3))
                nc.scalar.copy(zt[m][:], psZ[:])
                yield

            ht = [acp.tile([128, N], bf, tag=f"ht{j}", name=f"ht{j}")
                  for j in range(NFH)]
            for j in range(NFH):
                psH = psp.tile([128, N], fp, tag="mm", name="psH")
                for c in range(NCH):
                    nc.tensor.matmul(psH[:], w1t[0][c][:, j * 128:(j + 1) * 128],
                                     xt[c][:, :N], start=(c == 0), stop=False)
                for c in range(NCH):
                    nc.tensor.matmul(psH[:], w1t[1][c][:, j * 128:(j + 1) * 128],
                                     zt[c][:], start=False, stop=(c == NCH - 1))
                nc.scalar.activation(ht[j][:], psH[:], AF.Relu,
                                     bias=b1t[j][:], scale=1.0)
                yield

            qt = [acp.tile([128, N], bf, tag=f"qt{j}", name=f"qt{j}")
                  for j in range(NFH)]
            for j in range(NFH):
                psQ = psp.tile([128, N], fp, tag="mm", name="psQ")
                hbs = []
                for t in range(4):
                    psT2 = pst.tile([128, 128], bf, tag="psT", name="psT2")
                    nc.tensor.transpose(psT2[:],
                                        ht[j][:, t * 128:(t + 1) * 128],
                                        ident[:])
                    hb = bnd.tile([128, 128], bf, tag="hb", bufs=5, name="hb")
                    nc.scalar.copy(hb[:], psT2[:])
                    hbs.append(hb)
                for t in range(4):
                    nc.tensor.matmul(psQ[:], hbs[t][:], at[t][:],
                                     start=(t == 0), stop=(t == 3))
                nc.scalar.copy(qt[j][:], psQ[:])
                yield

            for m in range(NCH):
                psO = psp.tile([128, N], fp, tag="mm", name="psO")
                for j in range(NFH):
                    nc.tensor.matmul(psO[:], w2t[0][j][:, m * 128:(m + 1) * 128],
                                     ht[j][:], start=(j == 0), stop=False)
                for j in range(NFH):
                    nc.tensor.matmul(psO[:], w2t[1][j][:, m * 128:(m + 1) * 128],
                                     qt[j][:], start=False, stop=(j == NFH - 1))
                ot = otp.tile([128, N], fp, tag="ot", name="ot")
                nc.scalar.activation(ot[:], psO[:], AF.Relu,
                                     bias=b2t[m][:], scale=1.0)
                nc.sync.dma_start(out_p[b, m * 128:(m + 1) * 128, :], ot[:])
                yield

        # band0 fully; then interleave band1 with cheb0 so ChebConv-0
        # matmuls fill PE slack while DVE paces band1; cheb1 last.
        for _ in gen_band(0):
            pass
        g_band1, g_cheb0 = gen_band(1), gen_cheb(0)
        done_b = done_c = False
        while not (done_b and done_c):
            for _ in range(3):
                if not done_b:
                    done_b = next(g_band1, StopIteration) is StopIteration
            if not done_c:
                done_c = next(g_cheb0, StopIteration) is StopIteration
        for _ in gen_cheb(1):
            pass

        if rep_cm is not None:
            rep_cm.__exit__(None, None, None)

    if not nc.is_finalized():
        nc.finalize()
    return nc


def _prepare(x4, attention):
    """Host prep: flatten, sort by attention, compute band width."""
    X = np.ascontiguousarray(x4.reshape(B, N, F), dtype=np.float32)
    att = np.ascontiguousarray(attention[:, :, 0, 0], dtype=np.float32)
    perms = np.argsort(att, axis=1, kind="stable")
    attp = np.take_along_axis(att, perms, axis=1)
    a64 = attp.astype(np.float64)
    w = 1
    for bi in range(B):
        for d in range(1, N):
            if np.min(a64[bi, d:] - a64[bi, :-d]) <= ATT_THRESH + 1e-6:
                w = max(w, d)
            else:
                break  # windows only widen with d
    w = min(w, N - 1)
    Xp = np.take_along_axis(X, perms[:, :, None], axis=1)
    return Xp, attp, perms, w


def _maybe_patch_ldw_opt():
    """Optionally enable walrus's redundant-LDWEIGHTS optimization."""
    import os
    if os.environ.get("KERNEL_LDW_OPT", "0") != "1":
        return
    from concourse import bass_utils
    if getattr(bass_utils, "_ldw_patched", False):
        return
    bass_utils._ldw_patched = True
    orig_run = bass_utils.run_command

    def run2(argv, **kw):
        argv = ["--enable-ldw-opt=true" if x == "--enable-ldw-opt=false" else x
                for x in argv]
        return orig_run(argv, **kw)

    bass_utils.run_command = run2


def _make_runner(nc):
    """Compile the Bass program into a reusable 8-core sharded jax callable."""
    import jax
    from jax.sharding import Mesh, PartitionSpec
    from jax.experimental.shard_map import shard_map
    from concourse import bass2jax, mybir

    _maybe_patch_ldw_opt()
    bass2jax.install_neuronx_cc_hook()

    in_names, out_names, out_avals, zero_outs = [], [], [], []
    partition_name = (nc.partition_id_tensor.name
                      if nc.partition_id_tensor else None)
    for alloc in nc.m.functions[0].allocations:
        if not isinstance(alloc, mybir.MemoryLocationSet):
            continue
        name = alloc.memorylocations[0].name
        if alloc.kind == "ExternalInput":
            if name != partition_name:
                in_names.append(name)
        elif alloc.kind == "ExternalOutput":
            shape = tuple(alloc.tensor_shape)
            dtype = mybir.dt.np(alloc.dtype)
            out_names.append(name)
            out_avals.append(jax.core.ShapedArray(shape, dtype))
            zero_outs.append(np.zeros(shape, dtype))
    n_params = len(in_names)
    n_outs = len(out_avals)
    in_names = in_names + out_names
    if partition_name is not None:
        in_names.append(partition_name)
    donate = tuple(range(n_params, n_params + n_outs))

    def _body(*args):
        operands = list(args)
        if partition_name is not None:
            operands.append(bass2jax.partition_id_tensor())
        outs = bass2jax._bass_exec_p.bind(
            *operands,
            out_avals=tuple(out_avals),
            in_names=tuple(in_names),
            out_names=tuple(out_names),
            lowering_input_output_aliases=(),
            sim_require_finite=True,
            sim_require_nnan=True,
            nc=nc,
        )
        return tuple(outs)

    devices = jax.devices()[:NCORES]
    mesh = Mesh(np.asarray(devices), ("core",))
    sharded = jax.jit(
        shard_map(_body, mesh=mesh,
                  in_specs=(PartitionSpec("core"),) * (n_params + n_outs),
                  out_specs=(PartitionSpec("core"),) * n_outs,
                  check_rep=False),
        donate_argnums=donate, keep_unused=True)

    param_order = in_names[:n_params]

    def run(in_maps):
        concat_in = [
            np.concatenate([np.asarray(in_maps[c][nm]) for c in range(NCORES)],
                           axis=0)
            for nm in param_order
        ]
        concat_zeros = [np.zeros((NCORES * z.shape[0], *z.shape[1:]), z.dtype)
                        for z in zero_outs]
        out_arrs = jax.block_until_ready(sharded(*concat_in, *concat_zeros))
        return [
            {nm: np.asarray(out_arrs[i]).reshape(NCORES, *out_avals[i].shape)[c]
             for i, nm in enumerate(out_names)}
            for c in range(NCORES)
        ]

    return {"run": run, "sharded": sharded, "param_order": param_order,
            "zero_outs": zero_outs, "out_names": out_names,
            "out_avals": out_avals, "mesh": mesh}


def _get_runner(w):
    import os
    reps = int(os.environ.get("KERNEL_REPS", "1"))
    key = (w, reps)
    if key not in _prog_cache:
        nc = _build_program(w, reps=reps)
        _prog_cache[key] = (_make_runner(nc), nc)
    return _prog_cache[key]


def kernel(x4, attention, W1, b1, W2, b2):
    from concourse import mybir
    bfnp = mybir.dt.np(mybir.dt.bfloat16)

    Xp, attp, perms, w = _prepare(x4, attention)
    runner, nc = _get_runner(w)

    xpt = np.zeros((B, FP, XTW), np.float32)
    xpt[:, :F, :N] = Xp.transpose(0, 2, 1)
    xpt = xpt.astype(bfnp)
    xp = np.zeros((B, N, FP), np.float32)
    xp[:, :, :F] = Xp
    xp = xp.astype(bfnp)
    attp_pad = np.full((B, XTW), 1e9, np.float32)
    attp_pad[:, :N] = attp

    w1 = np.zeros((2, FP, FHP), np.float32)
    w1[:, :F, :FH] = np.asarray(W1, np.float32)
    w1 = w1.astype(bfnp)
    w2 = np.zeros((2, FHP, FP), np.float32)
    w2[:, :FH, :F] = np.asarray(W2, np.float32)
    w2 = w2.astype(bfnp)
    b1p = np.zeros((FHP,), np.float32)
    b1p[:FH] = np.asarray(b1, np.float32)
    b2p = np.zeros((FP,), np.float32)
    b2p[:F] = np.asarray(b2, np.float32)

    c_ones = np.ones((128, 1), np.float32).astype(bfnp)
    c_onesrow = np.ones((1, N), np.float32).astype(bfnp)
    c_estep = np.zeros((128, 128), np.float32)
    c_estep[:, 63] = 1.0
    c_estep = c_estep.astype(bfnp)
    # transpose permutation: combF row q<w holds mask d=q+1 -> band col
    # w+d; row 63 diagonal -> col w; row 64+q holds shifted mask d=q+1
    # -> band col w-1-q; all other rows map to nothing (zero).
    c_perm = np.zeros((128, 128), np.float32)
    for q in range(w):
        c_perm[q, w + q + 1] = 1.0
        c_perm[64 + q, w - 1 - q] = 1.0
    c_perm[63, w] = 1.0
    c_perm = c_perm.astype(bfnp)

    in_maps = []
    for c in range(NCORES):
        sl = slice(c * SPB, (c + 1) * SPB)
        in_maps.append({
            "xpt": np.ascontiguousarray(xpt[sl]),
            "xp": np.ascontiguousarray(xp[sl]),
            "attp": np.ascontiguousarray(attp_pad[sl]),
            "w1": w1, "b1": b1p, "w2": w2, "b2": b2p,
            "c_ones": c_ones, "c_onesrow": c_onesrow,
            "c_estep": c_estep, "c_perm": c_perm,
        })

    results = runner["run"](in_maps)
    globals()["last_in_maps"] = in_maps
    globals()["last_runner"] = runner
    globals()["last_nc"] = nc

    inv = np.argsort(perms, axis=1)
    out = np.empty((B, N, F), np.float32)
    for c in range(NCORES):
        o = results[c]["outT"]  # [SPB, FP, N]
        for s in range(SPB):
            bi = c * SPB + s
            out[bi] = o[s, :F, :].T[inv[bi]]
    return out


# revision 46
# speedup vs baseline: 305.0939x; 1.0122x over previous
"""Trainium2 Bass kernel: batched ChebConv GNN with L1-distance adjacency.

Pipeline per sample (N=512 nodes, F=625 features, padded to 640):
  1. Sort nodes by attention (host). All pairs with |att_i-att_j| <= 0.05
     then lie within a rank band |i-j| <= w (w computed exactly on host).
  2. Banded pairwise L1 distances on device via the identity
     sum_f |a-b| = 2*sum_f max(a,b) - S_i - S_j, computed with bf16 DVE
     max ops + PE ones-selector matmuls for the feature reduction.
  3. Threshold masks -> banded adjacency rows [2w+1, N] in SBUF. The
     negative offsets are obtained by a skewed DRAM round trip (per-row
     shift = row-granular diagonal AP, contiguous 512-element runs); the
     offset-axis reversal is folded into the PE transpose by using a
     permutation matrix instead of the identity. The transposed band is
     written to a skewed DRAM layout As2[r, (r%128)+k] so that dense
     adjacency tiles [128, 512] load back as plain constant-offset rows.
  4. Degree-normalized ChebConv x2 as bf16 PE matmuls in transposed
     layouts; the 1/deg column scaling is applied in fp32 during the
     PSUM->SBUF copies (DVE multiply against a broadcast dinv tile).
Data parallel over batch: 16 samples, 8 cores, 2 samples/core.
"""

import numpy as np
from contextlib import ExitStack

B, N = 16, 512
F, FP = 625, 640          # features, padded (5 x 128)
FH, FHP = 937, 1024       # hidden features, padded (8 x 128)
NCH, NFH = 5, 8           # 128-chunks of FP / FHP
NCORES = 8
SPB = B // NCORES         # samples per core
DIST_THRESH, ATT_THRESH = 180.0, 0.05
XTW = 560                 # xt row width: N + w + pad
PM = 608                  # msh scratch row pitch (>= 512 + w + 1)
PA = 216                  # As2 row pitch (>= 127 + 2w+1 - 127 ... >= 212)
BD = 4                    # band offsets per DVE instruction

_prog_cache = {}


def _build_program(w, reps=1):
    """Build the SPMD Bass program for band half-width w (<= 63)."""
    import concourse.bass as bass
    import concourse.bacc as bacc
    import concourse.mybir as mybir
    import concourse.tile as tile
    from concourse.masks import make_identity

    assert 1 <= w <= 63, w
    KW = 2 * w + 1

    dt = mybir.dt
    fp = dt.float32
    bf = dt.bfloat16
    AF = mybir.ActivationFunctionType
    OP = mybir.AluOpType
    AP = bass.AP

    nc = bacc.Bacc()
    xpt_p = nc.declare_dram_parameter("xpt", [SPB, FP, XTW], bf, isOutput=False)
    xp_p = nc.declare_dram_parameter("xp", [SPB, N, FP], bf, isOutput=False)
    attp_p = nc.declare_dram_parameter("attp", [SPB, XTW], fp, isOutput=False)
    w1_p = nc.declare_dram_parameter("w1", [2, FP, FHP], bf, isOutput=False)
    b1_p = nc.declare_dram_parameter("b1", [FHP], fp, isOutput=False)
    w2_p = nc.declare_dram_parameter("w2", [2, FHP, FP], bf, isOutput=False)
    b2_p = nc.declare_dram_parameter("b2", [FP], fp, isOutput=False)
    ones_p = nc.declare_dram_parameter("c_ones", [128, 1], bf, isOutput=False)
    onesrow_p = nc.declare_dram_parameter("c_onesrow", [1, N], bf, isOutput=False)
    estep_p = nc.declare_dram_parameter("c_estep", [128, 128], bf, isOutput=False)
    perm_p = nc.declare_dram_parameter("c_perm", [128, 128], bf, isOutput=False)
    out_p = nc.declare_dram_parameter("outT", [SPB, FP, N], bf, isOutput=True)

    # internal DRAM scratch, one set per sample slot
    as2 = [nc.dram_tensor(f"as2_{b}", [4 * 128 * PA], bf) for b in range(SPB)]
    msh = [nc.dram_tensor(f"msh_{b}", [128 * PA], bf) for b in range(SPB)]
    s_scr = [nc.dram_tensor(f"s_scr{b}", [XTW], fp) for b in range(SPB)]
    d_scr = [nc.dram_tensor(f"d_scr{b}", [N], bf) for b in range(SPB)]

    with tile.TileContext(nc) as tc, ExitStack() as ctx:
        cst = ctx.enter_context(tc.tile_pool(name="cst", bufs=1))
        xtp = ctx.enter_context(tc.tile_pool(name="xtp", bufs=1))
        xnp = ctx.enter_context(tc.tile_pool(name="xnp", bufs=1))
        mxp = ctx.enter_context(tc.tile_pool(name="mxp", bufs=2))
        bnd = ctx.enter_context(tc.tile_pool(name="bnd", bufs=1))
        amp = ctx.enter_context(tc.tile_pool(name="amp", bufs=1))
        acp = ctx.enter_context(tc.tile_pool(name="acp", bufs=1))
        wsp = ctx.enter_context(tc.tile_pool(name="wsp", bufs=1))
        otp = ctx.enter_context(tc.tile_pool(name="otp", bufs=2))
        psp = ctx.enter_context(tc.tile_pool(name="psp", bufs=2, space="PSUM"))
        psm = ctx.enter_context(tc.tile_pool(name="psm", bufs=2, space="PSUM"))
        ps1 = ctx.enter_context(tc.tile_pool(name="ps1", bufs=1, space="PSUM"))
        pst = ctx.enter_context(tc.tile_pool(name="pst", bufs=2, space="PSUM"))

        # ---- constants / weights resident in SBUF
        ones = cst.tile([128, 1], bf, tag="ones")
        nc.scalar.dma_start(ones[:], ones_p[:, :])
        estep = cst.tile([128, 128], bf, tag="estep")
        nc.scalar.dma_start(estep[:], estep_p[:, :])
        perm = cst.tile([128, 128], bf, tag="perm")
        nc.scalar.dma_start(perm[:], perm_p[:, :])
        ident = cst.tile([128, 128], bf, tag="ident")
        make_identity(nc, ident[:])
        zsrc = cst.tile([128, PA], bf, tag="zsrc")
        nc.gpsimd.memset(zsrc[:], 0.0)

        # weights go on the Activation HWDGE queue so they don't delay the
        # sample-0 input loads issued on the Sync queue by gen_band(0).
        w1t = [[wsp.tile([128, FHP], bf, tag=f"w1t{k}{c}", name=f"w1t{k}{c}")
                for c in range(NCH)] for k in range(2)]
        for k in range(2):
            for c in range(NCH):
                nc.scalar.dma_start(w1t[k][c][:],
                                    w1_p[k, c * 128:(c + 1) * 128, :])
        w2t = [[wsp.tile([128, FP], bf, tag=f"w2t{k}{j}", name=f"w2t{k}{j}")
                for j in range(NFH)] for k in range(2)]
        for k in range(2):
            for j in range(NFH):
                nc.scalar.dma_start(w2t[k][j][:],
                                    w2_p[k, j * 128:(j + 1) * 128, :])
        b1t = [wsp.tile([128, 1], fp, tag=f"b1t{j}", name=f"b1t{j}")
               for j in range(NFH)]
        for j in range(NFH):
            nc.scalar.dma_start(b1t[j][:], b1_p[j * 128:(j + 1) * 128])
        b2t = [wsp.tile([128, 1], fp, tag=f"b2t{m}", name=f"b2t{m}")
               for m in range(NCH)]
        for m in range(NCH):
            nc.scalar.dma_start(b2t[m][:], b2_p[m * 128:(m + 1) * 128])

        # per-sample persistent tiles (zeroed once; geometry-stable reuse)
        combF = [cst.tile([128, 512], bf, tag=f"combF{b}", name=f"combF{b}")
                 for b in range(SPB)]
        at_t = [[amp.tile([128, 512], bf, tag=f"at{b}{t}", name=f"at{b}{t}")
                 for t in range(4)] for b in range(SPB)]
        dinvB = [bnd.tile([128, 512], bf, tag=f"dinvB{b}", name=f"dinvB{b}")
                 for b in range(SPB)]
        for b in range(SPB):
            nc.gpsimd.memset(combF[b][:], 0.0)
            # diagonal (identity) row of the band lives at partition 63;
            # engine ops can't target unaligned partition bases, DMA can.
            nc.scalar.dma_start(combF[b][63:64, :N], onesrow_p[:, :])
            for t in range(4):
                nc.gpsimd.memset(at_t[b][t][:], 0.0)
            # zero-fill DRAM scratch (guard zeros; geometry-stable after)
            for t in range(4):
                nc.scalar.dma_start(AP(as2[b], t * 128 * PA, [[1, 128 * PA]]),
                                    zsrc[:])
            nc.scalar.dma_start(AP(msh[b], 0, [[1, 128 * PA]]), zsrc[:])
            nc.scalar.dma_start(AP(s_scr[b], N, [[1, XTW - N]]),
                                zsrc[0:1, 0:2 * (XTW - N)].bitcast(fp))

        # PE warmup: ~16 x 216-cycle matmuls on zeros keeps HAM busy while
        # the input DMAs land, so the band matmuls start at full clock.
        psW = psm.tile([w, N], fp, tag="psM", name="psW")
        for i in range(16):
            nc.tensor.matmul(psW[:, :216], ident[:, :w], zsrc[:, :216],
                             start=(i == 0), stop=(i == 15))

        rep_cm = tc.For_i(0, reps, 1) if reps > 1 else None
        if rep_cm is not None:
            rep_cm.__enter__()

        xt_all = [None] * SPB

        def gen_band(b):
            xt = [xtp.tile([128, XTW], bf, tag=f"xt{b}{c}", name=f"xt{b}{c}")
                  for c in range(NCH)]
            for c in range(NCH):
                nc.sync.dma_start(xt[c][:], xpt_p[b, c * 128:(c + 1) * 128, :])
            xt_all[b] = xt
            # S row sums via ones matmul (fp32 accumulate)
            psS = ps1.tile([1, N], fp, tag="ps1", name="psS")
            for c in range(NCH):
                nc.tensor.matmul(psS[:], ones[:], xt[c][:, :N],
                                 start=(c == 0), stop=(c == NCH - 1))
            srow = bnd.tile([1, N], fp, tag="srow", name="srow")
            nc.scalar.copy(srow[:], psS[:])
            nc.sync.dma_start(AP(s_scr[b], 0, [[1, N]]), srow[:])

            # banded max sums: psM[di, i] = sum_f max(x[i,f], x[i+di+1,f])
            psM = psm.tile([w, N], fp, tag="psM", name="psM")
            nbat = (w + BD - 1) // BD
            for bi in range(nbat):
                db0 = bi * BD
                nb = min(BD, w - db0)
                mxb = [mxp.tile([128, BD * N], bf, tag=f"mx{c}",
                                name=f"mxb{c}") for c in range(NCH)]
                for c in range(NCH):
                    base = xt[c][:, 0:N]
                    in0 = bass.AP(base.tensor, base.offset,
                                  [list(base.ap[0]), [0, nb], [1, N]])
                    in1 = bass.AP(base.tensor, base.offset + 1 + db0,
                                  [list(base.ap[0]), [1, nb], [1, N]])
                    nc.vector.tensor_tensor(
                        out=mxb[c][:, :nb * N], in0=in0, in1=in1, op=OP.max)
                # j-outer / c-inner: 5 consecutive matmuls share one
                # estep slice
                for j in range(nb):
                    di = db0 + j
                    for c in range(NCH):
                        nc.tensor.matmul(
                            psM[:, :], estep[:, 63 - di:63 - di + w],
                            mxb[c][:, j * N:(j + 1) * N],
                            start=(di == 0 and c == 0),
                            stop=(di == w - 1 and c == NCH - 1))
                    yield

            # thresholds -> positive-offset masks, written into combF rows
            # [0, w) (ascending d); then skewed round trip builds the
            # shifted masks in rows [64, 64+w).
            sshift = bnd.tile([w, N], fp, tag="sshift", name="sshift")
            nc.sync.dma_start(sshift[:], AP(s_scr[b], 1, [[1, w], [1, N]]))
            sb_t = bnd.tile([w, N], fp, tag="sb", name="sb_t")
            nc.sync.dma_start(sb_t[:], AP(s_scr[b], 0, [[0, w], [1, N]]))
            ashift = bnd.tile([w, N], fp, tag="ashift", name="ashift")
            nc.sync.dma_start(ashift[:],
                              AP(attp_p, b * XTW + 1, [[1, w], [1, N]]))
            ab_t = bnd.tile([w, N], fp, tag="ab", name="ab_t")
            nc.sync.dma_start(ab_t[:], AP(attp_p, b * XTW, [[0, w], [1, N]]))
            nc.vector.scalar_tensor_tensor(
                out=sb_t[:], in0=sb_t[:], scalar=DIST_THRESH, in1=sshift[:],
                op0=OP.add, op1=OP.add)
            nc.vector.scalar_tensor_tensor(
                out=sshift[:], in0=psM[:], scalar=2.0, in1=sb_t[:],
                op0=OP.mult, op1=OP.is_le)
            nc.vector.tensor_sub(ashift[:], ashift[:], ab_t[:])
            nc.vector.tensor_scalar(ab_t[:], ashift[:], ATT_THRESH, None,
                                    op0=OP.is_le)
            nc.vector.tensor_mul(combF[b][0:w, :N],
                                 sshift[:], ab_t[:])
            yield
            # skewed write: msh[p, (p+1)+x] = mask_{p+1}[x]
            nc.sync.dma_start(AP(msh[b], 1, [[PM + 1, w], [1, N]]),
                              combF[b][0:w, :N])
            # plain read: combF[64+q, i] = msh[q, i] = mask_{q+1}[i-(q+1)]
            nc.sync.dma_start(combF[b][64:64 + w, :N],
                              AP(msh[b], 0, [[PM, w], [1, N]]))
            # degree from all band rows + diagonal ones row
            psD = ps1.tile([1, N], fp, tag="ps1", name="psD")
            nc.tensor.matmul(psD[:], ones[:], combF[b][:, :N],
                             start=True, stop=True)
            drow = bnd.tile([1, N], bf, tag="drow", name="drow")
            with nc.allow_low_precision(reason="bf16 1/deg column scale"):
                nc.vector.reciprocal(drow[:], psD[:])
            nc.sync.dma_start(AP(d_scr[b], 0, [[1, N]]), drow[:])
            nc.sync.dma_start(dinvB[b][:], AP(d_scr[b], 0, [[0, 128], [1, N]]))
            yield
            # permuted transpose -> skewed As2 write -> dense at tiles
            for t in range(4):
                psT = pst.tile([128, 128], bf, tag="psT", name="psT")
                nc.tensor.transpose(psT[:],
                                    combF[b][:, t * 128:(t + 1) * 128],
                                    perm[:])
                ct = bnd.tile([128, KW], bf, tag="ct", bufs=2, name="ct")
                nc.scalar.copy(ct[:], psT[:, :KW])
                nc.sync.dma_start(
                    AP(as2[b], t * 128 * PA, [[PA + 1, 128], [1, KW]]),
                    ct[:])
                yield
            for t in range(4):
                o0 = t * 128 - w
                c0 = max(0, o0)
                c1 = min(N, o0 + KW + 127)
                m0 = c0 - o0
                nc.sync.dma_start(
                    at_t[b][t][:, c0:c1],
                    AP(as2[b], t * 128 * PA + m0, [[PA, 128], [1, c1 - c0]]))
                # fold the 1/deg column scaling into the adjacency tiles
                # (bf16 dinv; zt/qt copies then run on the idle ACT engine)
                nc.vector.tensor_mul(at_t[b][t][:, c0:c1],
                                     at_t[b][t][:, c0:c1],
                                     dinvB[b][:, c0:c1])
            yield

        def gen_cheb(b):
            xt, at = xt_all[b], at_t[b]
            xn = [xnp.tile([128, FP], bf, tag=f"xn{b}{t}", name=f"xn{b}{t}")
                  for t in range(4)]
            for t in range(4):
                nc.sync.dma_start(xn[t][:], xp_p[b, t * 128:(t + 1) * 128, :])

            zt = [acp.tile([128, N], bf, tag=f"zt{m}", name=f"zt{m}")
                  for m in range(NCH)]
            for m in range(NCH):
                psZ = psp.tile([128, N], fp, tag="mm", name="psZ")
                for t in range(4):
                    nc.tensor.matmul(psZ[:], xn[t][:, m * 128:(m + 1) * 128],
                                     at[t][:], start=(t == 0), stop=(t == 3))
                nc.scalar.copy(zt[m][:], psZ[:])
                yield

            ht = [acp.tile([128, N], bf, tag=f"ht{j}", name=f"ht{j}")
                  for j in range(NFH)]
            for j in range(NFH):
                psH = psp.tile([128, N], fp, tag="mm", name="psH")
                for c in range(NCH):
                    nc.tensor.matmul(psH[:], w1t[0][c][:, j * 128:(j + 1) * 128],
                                     xt[c][:, :N], start=(c == 0), stop=False)
                for c in range(NCH):
                    nc.tensor.matmul(psH[:], w1t[1][c][:, j * 128:(j + 1) * 128],
                                     zt[c][:], start=False, stop=(c == NCH - 1))
                nc.scalar.activation(ht[j][:], psH[:], AF.Relu,
                                     bias=b1t[j][:], scale=1.0)
                yield

            qt = [acp.tile([128, N], bf, tag=f"qt{j}", name=f"qt{j}")
                  for j in range(NFH)]
            for j in range(NFH):
                psQ = psp.tile([128, N], fp, tag="mm", name="psQ")
                hbs = []
                for t in range(4):
                    psT2 = pst.tile([128, 128], bf, tag="psT", name="psT2")
                    nc.tensor.transpose(psT2[:],
                                        ht[j][:, t * 128:(t + 1) * 128],
                                        ident[:])
                    hb = bnd.tile([128, 128], bf, tag="hb", bufs=5, name="hb")
                    nc.scalar.copy(hb[:], psT2[:])
                    hbs.append(hb)
                for t in range(4):
                    nc.tensor.matmul(psQ[:], hbs[t][:], at[t][:],
                                     start=(t == 0), stop=(t == 3))
                nc.scalar.copy(qt[j][:], psQ[:])
                yield

            for m in range(NCH):
                psO = psp.tile([128, N], fp, tag="mm", name="psO")
                for j in range(NFH):
                    nc.tensor.matmul(psO[:], w2t[0][j][:, m * 128:(m + 1) * 128],
                                     ht[j][:], start=(j == 0), stop=False)
                for j in range(NFH):
                    nc.tensor.matmul(psO[:], w2t[1][j][:, m * 128:(m + 1) * 128],
                                     qt[j][:], start=False, stop=(j == NFH - 1))
                ot = otp.tile([128, N], bf, tag="ot", name="ot")
                nc.scalar.activation(ot[:], psO[:], AF.Relu,
                                     bias=b2t[m][:], scale=1.0)
                nc.sync.dma_start(out_p[b, m * 128:(m + 1) * 128, :], ot[:])
                yield

        # band0 fully; then interleave band1 with cheb0 so ChebConv-0
        # matmuls fill PE slack while DVE paces band1; cheb1 last.
        for _ in gen_band(0):
            pass
        g_band1, g_cheb0 = gen_band(1), gen_cheb(0)
        done_b = done_c = False
        while not (done_b and done_c):
            for _ in range(3):
                if not done_b:
                    done_b = next(g_band1, StopIteration) is StopIteration
            if not done_c:
                done_c = next(g_cheb0, StopIteration) is StopIteration
        for _ in gen_cheb(1):
            pass

        if rep_cm is not None:
            rep_cm.__exit__(None, None, None)

    if not nc.is_finalized():
        nc.finalize()
    return nc


def _prepare(x4, attention):
    """Host prep: flatten, sort by attention, compute band width."""
    X = np.ascontiguousarray(x4.reshape(B, N, F), dtype=np.float32)
    att = np.ascontiguousarray(attention[:, :, 0, 0], dtype=np.float32)
    perms = np.argsort(att, axis=1, kind="stable")
    attp = np.take_along_axis(att, perms, axis=1)
    a64 = attp.astype(np.float64)
    w = 1
    for bi in range(B):
        for d in range(1, N):
            if np.min(a64[bi, d:] - a64[bi, :-d]) <= ATT_THRESH + 1e-6:
                w = max(w, d)
            else:
                break  # windows only widen with d
    w = min(w, N - 1)
    Xp = np.take_along_axis(X, perms[:, :, None], axis=1)
    return Xp, attp, perms, w


def _maybe_patch_ldw_opt():
    """Optionally enable walrus's redundant-LDWEIGHTS optimization."""
    import os
    if os.environ.get("KERNEL_LDW_OPT", "0") != "1":
        return
    from concourse import bass_utils
    if getattr(bass_utils, "_ldw_patched", False):
        return
    bass_utils._ldw_patched = True
    orig_run = bass_utils.run_command

    def run2(argv, **kw):
        argv = ["--enable-ldw-opt=true" if x == "--enable-ldw-opt=false" else x
                for x in argv]
        return orig_run(argv, **kw)

    bass_utils.run_command = run2


def _make_runner(nc):
    """Compile the Bass program into a reusable 8-core sharded jax callable."""
    import jax
    from jax.sharding import Mesh, PartitionSpec
    from jax.experimental.shard_map import shard_map
    from concourse import bass2jax, mybir

    _maybe_patch_ldw_opt()
    bass2jax.install_neuronx_cc_hook()

    in_names, out_names, out_avals, zero_outs = [], [], [], []
    partition_name = (nc.partition_id_tensor.name
                      if nc.partition_id_tensor else None)
    for alloc in nc.m.functions[0].allocations:
        if not isinstance(alloc, mybir.MemoryLocationSet):
            continue
        name = alloc.memorylocations[0].name
        if alloc.kind == "ExternalInput":
            if name != partition_name:
                in_names.append(name)
        elif alloc.kind == "ExternalOutput":
            shape = tuple(alloc.tensor_shape)
            dtype = mybir.dt.np(alloc.dtype)
            out_names.append(name)
            out_avals.append(jax.core.ShapedArray(shape, dtype))
            zero_outs.append(np.zeros(shape, dtype))
    n_params = len(in_names)
    n_outs = len(out_avals)
    in_names = in_names + out_names
    if partition_name is not None:
        in_names.append(partition_name)
    donate = tuple(range(n_params, n_params + n_outs))

    def _body(*args):
        operands = list(args)
        if partition_name is not None:
            operands.append(bass2jax.partition_id_tensor())
        outs = bass2jax._bass_exec_p.bind(
            *operands,
            out_avals=tuple(out_avals),
            in_names=tuple(in_names),
            out_names=tuple(out_names),
            lowering_input_output_aliases=(),
            sim_require_finite=True,
            sim_require_nnan=True,
            nc=nc,
        )
        return tuple(outs)

    devices = jax.devices()[:NCORES]
    mesh = Mesh(np.asarray(devices), ("core",))
    sharded = jax.jit(
        shard_map(_body, mesh=mesh,
                  in_specs=(PartitionSpec("core"),) * (n_params + n_outs),
                  out_specs=(PartitionSpec("core"),) * n_outs,
                  check_rep=False),
        donate_argnums=donate, keep_unused=True)

    param_order = in_names[:n_params]

    def run(in_maps):
        concat_in = [
            np.concatenate([np.asarray(in_maps[c][nm]) for c in range(NCORES)],
                           axis=0)
            for nm in param_order
        ]
        concat_zeros = [np.zeros((NCORES * z.shape[0], *z.shape[1:]), z.dtype)
                        for z in zero_outs]
        out_arrs = jax.block_until_ready(sharded(*concat_in, *concat_zeros))
        return [
            {nm: np.asarray(out_arrs[i]).reshape(NCORES, *out_avals[i].shape)[c]
             for i, nm in enumerate(out_names)}
            for c in range(NCORES)
        ]

    return {"run": run, "sharded": sharded, "param_order": param_order,
            "zero_outs": zero_outs, "out_names": out_names,
            "out_avals": out_avals, "mesh": mesh}


def _get_runner(w):
    import os
    reps = int(os.environ.get("KERNEL_REPS", "1"))
    key = (w, reps)
    if key not in _prog_cache:
        nc = _build_program(w, reps=reps)
        _prog_cache[key] = (_make_runner(nc), nc)
    return _prog_cache[key]


def kernel(x4, attention, W1, b1, W2, b2):
    from concourse import mybir
    bfnp = mybir.dt.np(mybir.dt.bfloat16)

    Xp, attp, perms, w = _prepare(x4, attention)
    runner, nc = _get_runner(w)

    xpt = np.zeros((B, FP, XTW), np.float32)
    xpt[:, :F, :N] = Xp.transpose(0, 2, 1)
    xpt = xpt.astype(bfnp)
    xp = np.zeros((B, N, FP), np.float32)
    xp[:, :, :F] = Xp
    xp = xp.astype(bfnp)
    attp_pad = np.full((B, XTW), 1e9, np.float32)
    attp_pad[:, :N] = attp

    w1 = np.zeros((2, FP, FHP), np.float32)
    w1[:, :F, :FH] = np.asarray(W1, np.float32)
    w1 = w1.astype(bfnp)
    w2 = np.zeros((2, FHP, FP), np.float32)
    w2[:, :FH, :F] = np.asarray(W2, np.float32)
    w2 = w2.astype(bfnp)
    b1p = np.zeros((FHP,), np.float32)
    b1p[:FH] = np.asarray(b1, np.float32)
    b2p = np.zeros((FP,), np.float32)
    b2p[:F] = np.asarray(b2, np.float32)

    c_ones = np.ones((128, 1), np.float32).astype(bfnp)
    c_onesrow = np.ones((1, N), np.float32).astype(bfnp)
    c_estep = np.zeros((128, 128), np.float32)
    c_estep[:, 63] = 1.0
    c_estep = c_estep.astype(bfnp)
    # transpose permutation: combF row q<w holds mask d=q+1 -> band col
    # w+d; row 63 diagonal -> col w; row 64+q holds shifted mask d=q+1
    # -> band col w-1-q; all other rows map to nothing (zero).
    c_perm = np.zeros((128, 128), np.float32)
    for q in range(w):
        c_perm[q, w + q + 1] = 1.0
        c_perm[64 + q, w - 1 - q] = 1.0
    c_perm[63, w] = 1.0
    c_perm = c_perm.astype(bfnp)

    in_maps = []
    for c in range(NCORES):
        sl = slice(c * SPB, (c + 1) * SPB)
        in_maps.append({
            "xpt": np.ascontiguousarray(xpt[sl]),
            "xp": np.ascontiguousarray(xp[sl]),
            "attp": np.ascontiguousarray(attp_pad[sl]),
            "w1": w1, "b1": b1p, "w2": w2, "b2": b2p,
            "c_ones": c_ones, "c_onesrow": c_onesrow,
            "c_estep": c_estep, "c_perm": c_perm,
        })

    results = runner["run"](in_maps)
    globals()["last_in_maps"] = in_maps
    globals()["last_runner"] = runner
    globals()["last_nc"] = nc

    inv = np.argsort(perms, axis=1)
    out = np.empty((B, N, F), np.float32)
    for c in range(NCORES):
        o = np.asarray(results[c]["outT"], dtype=np.float32)  # [SPB, FP, N]
        for s in range(SPB):
            bi = c * SPB + s
            out[bi] = o[s, :F, :].T[inv[bi]]
    return out
